# revision 1
# baseline (speedup 1.0000x reference)
"""Trainium2 Bass kernel for the deep-hedging Milstein SDE loss.

Math: the reference scan has closed-form structure. With y = [s, v]:
  s_{n+1} = s_n * m_n,  m_n = 1 + MU*dt + SIG*dW_n + 0.5*SIG^2*(dW_n^2 - dt)
  v_{n+1} = v_n + dhdt*dt + dhds*(s_{n+1}-s_n) + 0.5*SIG^2*s_n^2*dW_n^2*dhdss
where (dhdt, dhds, dhdss) are derivatives of the holding MLP h(t, s) at
(t_n, s_n).  So the scan collapses to:
  1. prefix-product along steps for s_n (tensor_tensor_scan)
  2. one fully-batched forward-mode jet evaluation of the MLP over all
     B*N points with 3 tangent streams (combined first-order gamma =
     ds-weighted + dt-weighted tangent, sqrt(c)-scaled s-tangent u, and
     c-scaled 2nd-order stream v)
  3. a per-path reduction over steps.

Layout per core (1024 paths):
  sgrid [128 part = p, 8 blocks b, 128 steps n], path_local = b*128 + p.
  MLP packs 4 groups of 32 features on partitions (group g = quartile
  g = p' block of paths); group point column j = (p'*8 + b)*128 + n.

Elementwise per hidden layer uses only the derivative_silu_and_others
ACT table set (Derivative_silu, Tanh, Square); silu itself is computed
on DVE as (z+b)*sigma with sigma = 0.5*tanh(z/2)+0.5, and
silu''(z) = sigma - silu'(z)*tanh(z/2).
"""

import numpy as np

import concourse.bass as bass
import concourse.mybir as mybir
from concourse import tile
from concourse.bass_utils import run_bass_kernel_spmd


# problem constants (hardcoded per spec)
B = 8192
NSTEP = 128
NCORE = 8
BC = B // NCORE          # 1024 paths per core
P = 128                  # partitions
NB = BC // P             # 8 path blocks
WIDTH = 32
NG = 4                   # feature groups on partitions
NH = 3                   # hidden layers
C = BC * NSTEP // NG     # 32768 point-columns per group
CC = 512                 # chunk columns
NCHUNK = C // CC         # 64
T0, T1 = 0.0, 1.0
MU, SIG = 1.0, 1.0
DT = (T1 - T0) / NSTEP
SQDT = float(np.sqrt(DT))

F32 = mybir.dt.float32
AF = mybir.ActivationFunctionType
ALU = mybir.AluOpType

# stream dtype for hidden activations / tangents (flip to bfloat16 for speed)
SD = mybir.dt.float16

_CACHE = {}
import os
DBG_STAGE = os.environ.get("KDBG_STAGE", "full")
DBG_NCHUNK = int(os.environ.get("KDBG_NCHUNK", "0")) or None
DBG_REPS = int(os.environ.get("KDBG_REPS", "1"))


def _legalize_waits(nc):
    """Split long on_wait lists into standalone single/dual-wait NoOps.

    This walrus rejects instructions whose sync_info carries more waits
    than the ISA encoding holds (1 for DMA descriptors, ~2 for compute /
    ctrl ops). Tile emits up to one wait per logical processor (27), so
    spill the excess onto NoOps on the same engine queue, which execute
    in order before the real instruction.
    """
    ctr = 0
    for bb in nc.main_func.blocks:
        out = []
        for ins in bb.instructions:
            si = ins.sync_info
            if si is not None and si.on_wait:
                # this walrus encodes exactly one sync wait per instruction
                limit = 1
                waits = list(si.on_wait)
                if len(waits) > limit:
                    spill, keep = waits[:-limit], waits[-limit:]
                    for w in spill:
                        ctr += 1
                        nop = mybir.InstNoOp(
                            name=f"waitnop_{ctr}", ins=[], outs=[]
                        )
                        nop.engine = ins.engine
                        nop.sync_info = mybir.SyncInfo(on_wait=[w], on_update=[])
                        out.append(nop)
                    si.on_wait = keep
            out.append(ins)
        bb.instructions = out


def _build_program():
    nc = bass.Bass()

    # ---- dram parameters (per-core inputs) ----
    rn_d = nc.declare_dram_parameter("rn_sg", [P, NB * NSTEP], F32, isOutput=False)
    trow_d = nc.declare_dram_parameter("trow", [P, 2, C // P], SD, isOutput=False)
    lhsT0_d = nc.declare_dram_parameter("lhsT0", [5 * NG, P], SD, isOutput=False)
    lhsTg_d = nc.declare_dram_parameter("lhsTg", [5 * NG, P], SD, isOutput=False)
    lhsTu_d = nc.declare_dram_parameter("lhsTu", [5 * NG, P], SD, isOutput=False)
    lhsTh_d = nc.declare_dram_parameter("lhsTh", [NH, P, P], SD, isOutput=False)
    lhsTf_d = nc.declare_dram_parameter("lhsTf", [P, NG], SD, isOutput=False)
    bias_d = nc.declare_dram_parameter("bias", [P, 4, 2], F32, isOutput=False)
    bfh_d = nc.declare_dram_parameter("bfh", [P, 1], F32, isOutput=False)
    out_d = nc.declare_dram_parameter("yT", [BC, 2], F32, isOutput=True)

    with tile.TileContext(nc) as tc:
        with (
            tc.tile_pool(name="const", bufs=1) as cpool,
            tc.tile_pool(name="sg", bufs=1) as sgpool,
            tc.tile_pool(name="work", bufs=2) as wpool,
            tc.tile_pool(name="psum", bufs=8, space="PSUM") as pspool,
        ):
          for _rep in range(DBG_REPS):
            # ---- load constants ----
              lhsT0 = cpool.tile([5 * NG, P], SD, tag="lhsT0")
              lhsTg = cpool.tile([5 * NG, P], SD, tag="lhsTg")
              lhsTu = cpool.tile([5 * NG, P], SD, tag="lhsTu")
              lhsTh = [
                  cpool.tile([P, P], SD, tag=f"lhsTh{l}", name=f"lhsTh{l}")
                  for l in range(NH)
              ]
              lhsTf = cpool.tile([P, NG], SD, tag="lhsTf")
              bias = cpool.tile([P, 4, 2], F32, tag="bias")
              bfh = cpool.tile([P, 1], F32, tag="bfh")
              nc.sync.dma_start(lhsT0[:], lhsT0_d[:])
              nc.sync.dma_start(lhsTg[:], lhsTg_d[:])
              nc.sync.dma_start(lhsTu[:], lhsTu_d[:])
              for l in range(NH):
                  nc.sync.dma_start(lhsTh[l][:], lhsTh_d[l])
              nc.sync.dma_start(lhsTf[:], lhsTf_d[:])
              nc.sync.dma_start(bias[:], bias_d[:])
              nc.sync.dma_start(bfh[:], bfh_d[:])
              # bias AP views [P, 1]: bias[:, l, 0] = b tiled, [:, l, 1] = 0.5*b
              def bias_r(l, h):
                  return bias[:, l, h : h + 1]

              # ---- stage A: sgrid GBM math ----
              rs = sgpool.tile([P, NB, NSTEP], F32, tag="rs")
              nc.sync.dma_start(rs[:], rn_d[:].rearrange("p (b n) -> p b n", b=NB))
              dW = sgpool.tile([P, NB, NSTEP], F32, tag="dW")
              nc.vector.tensor_scalar_mul(dW[:], rs[:], SQDT)
              m = sgpool.tile([P, NB, NSTEP], F32, tag="m")
              # q1 = dW^2 (reuse rs as scratch)
              nc.vector.tensor_mul(rs[:], dW[:], dW[:])
              # m = (q1 * 0.5*SIG^2) + SIG*dW   (SIG == 1)
              nc.vector.scalar_tensor_tensor(
                  m[:], rs[:], 0.5 * SIG * SIG, dW[:], ALU.mult, ALU.add
              )
              c0 = 1.0 + MU * DT - 0.5 * SIG * SIG * DT
              nc.vector.tensor_scalar_add(m[:], m[:], c0)

              # prefix product per block: sfull[:, b, 0] = 1; [:, b, 1+k] = prod
              sfull = sgpool.tile([P, NB, NSTEP + 1], F32, tag="sfull")
              nc.vector.memset(sfull[:, :, 0:1], 1.0)
              for b in range(NB):
                  nc.vector.tensor_tensor_scan(
                      sfull[:, b, 1 : NSTEP + 1],
                      m[:, b, :],
                      m[:, b, :],
                      1.0,
                      ALU.mult,
                      ALU.bypass,
                  )
              sN = sfull[:, :, 0:NSTEP]

              # Ds = (m - 1) * sN ; sdW = sN * dW   (bf16 copies for the rhs repack)
              Ds = sgpool.tile([P, NB, NSTEP], SD, tag="Ds")
              nc.vector.scalar_tensor_tensor(Ds[:], m[:], 1.0, sN, ALU.subtract, ALU.mult)
              sdW = sgpool.tile([P, NB, NSTEP], SD, tag="sdW")
              nc.vector.tensor_tensor(sdW[:], sN, dW[:], ALU.mult)
              sN_b = sgpool.tile([P, NB, NSTEP], SD, tag="sN_b")
              nc.vector.tensor_copy(sN_b[:], sN)

              run_B = DBG_STAGE in ("B", "C", "full")
              run_C = DBG_STAGE in ("C", "full")
              # ---- stage B: rhs0 assembly ----
              rhs0 = sgpool.tile([5 * NG, C], SD, tag="rhs0")
              trow = sgpool.tile([P, 2, C // P], SD, tag="trow")
              if not run_B:
                  nc.vector.memset(rhs0[0:1, 0:4], 0.0)
              if run_B:
                nc.sync.dma_start(trow[:], trow_d[:])
                for g in range(NG):
                  # t rows + ones rows: DMA-from-DMA (single queue dep)
                  dst = rhs0[5 * g : 5 * g + 1, :].rearrange(
                      "one (p c) -> one p c", p=P
                  )
                  nc.sync.dma_start(dst, trow[:, 0, :])
                  dst = rhs0[5 * g + 3 : 5 * g + 4, :].rearrange(
                      "one (p c) -> one p c", p=P
                  )
                  nc.sync.dma_start(dst, trow[:, 1, :])
                scr1 = cpool.tile([1, 4], SD, tag="scr1")
                nc.sync.dma_start(scr1[:, 0:1], rhs0[5 * (NG - 1) + 3 : 5 * (NG - 1) + 4, 0:1])
                for g in range(NG):
                  dst = rhs0[5 * g + 1 : 5 * g + 2, :].rearrange(
                      "one (q b n) -> one q b n", q=32, b=NB
                  )
                  nc.sync.dma_start(dst, sN_b[32 * g : 32 * (g + 1), :, :])
                  dst = rhs0[5 * g + 2 : 5 * g + 3, :].rearrange(
                      "one (q b n) -> one q b n", q=32, b=NB
                  )
                  nc.sync.dma_start(dst, Ds[32 * g : 32 * (g + 1), :, :])
                  dst = rhs0[5 * g + 4 : 5 * g + 5, :].rearrange(
                      "one (q b n) -> one q b n", q=32, b=NB
                  )
                  nc.sync.dma_start(dst, sdW[32 * g : 32 * (g + 1), :, :])
                # end run_B

              # ---- zf staging for stage D ----
              # zf_st[4s:4s+4, ci*CC:(ci+1)*CC] accumulates stream s chunks;
              # PSUM cannot be DMA'd, so chunks are copied out by DVE/ACT.
              zf_st = sgpool.tile([P, C], SD, tag="zf_st")  # stream s lives at partitions [32s, 32s+4)
              zf_sg = [
                  sgpool.tile([P, NB, NSTEP], SD, tag=f"zf_sg{s}", name=f"zf_sg{s}")
                  for s in range(4)
              ]

              # ---- stage C: chunked MLP jet evaluation ----
              nchunk = DBG_NCHUNK or NCHUNK
              for ci in (range(nchunk) if run_C else []):
                  rv = rhs0[:, ci * CC : (ci + 1) * CC]
                  Z0 = pspool.tile([P, CC], F32, tag="ps")
                  nc.tensor.matmul(Z0[:], lhsT0[:], rv, start=True, stop=True)
                  Mg = pspool.tile([P, CC], F32, tag="ps")
                  nc.tensor.matmul(Mg[:], lhsTg[:], rv, start=True, stop=True)
                  Mu = pspool.tile([P, CC], F32, tag="ps")
                  nc.tensor.matmul(Mu[:], lhsTu[:], rv, start=True, stop=True)

                  # layer-0 elementwise
                  s1 = wpool.tile([P, CC], SD, tag="s1")
                  nc.scalar.activation(
                      s1[:], Z0[:], AF.Derivative_silu, bias=bias_r(0, 0)
                  )
                  T = wpool.tile([P, CC], SD, tag="T")
                  nc.scalar.activation(
                      T[:], Z0[:], AF.Tanh, bias=bias_r(0, 1), scale=0.5
                  )
                  A = wpool.tile([P, CC], SD, tag="A")
                  nc.scalar.activation(A[:], Mu[:], AF.Square)
                  sig = wpool.tile([P, CC], SD, tag="sig")
                  nc.vector.tensor_scalar(sig[:], T[:], 0.5, 0.5, ALU.mult, ALU.add)
                  a = wpool.tile([P, CC], SD, tag="a")
                  nc.vector.scalar_tensor_tensor(
                      a[:], Z0[:], bias_r(0, 0), sig[:], ALU.add, ALU.mult
                  )
                  G = wpool.tile([P, CC], SD, tag="G")
                  nc.vector.tensor_tensor(G[:], s1[:], T[:], ALU.mult)
                  s2p = wpool.tile([P, CC], SD, tag="s2p")
                  nc.vector.scalar_tensor_tensor(
                      s2p[:], T[:], 0.5, G[:], ALU.mult, ALU.subtract
                  )
                  u = wpool.tile([P, CC], SD, tag="u")
                  nc.vector.tensor_tensor(u[:], s1[:], Mu[:], ALU.mult)
                  gm = wpool.tile([P, CC], SD, tag="gm")
                  nc.vector.tensor_tensor(gm[:], s1[:], Mg[:], ALU.mult)
                  v = wpool.tile([P, CC], SD, tag="v")
                  nc.vector.scalar_tensor_tensor(
                      v[:], s2p[:], 0.5, A[:], ALU.add, ALU.mult
                  )

                  # hidden layers
                  for l in range(NH):
                      Zp = pspool.tile([P, CC], F32, tag="ps")
                      nc.tensor.matmul(Zp[:], lhsTh[l][:], a[:], start=True, stop=True)
                      Zu = pspool.tile([P, CC], F32, tag="ps")
                      nc.tensor.matmul(Zu[:], lhsTh[l][:], u[:], start=True, stop=True)
                      Zg = pspool.tile([P, CC], F32, tag="ps")
                      nc.tensor.matmul(Zg[:], lhsTh[l][:], gm[:], start=True, stop=True)
                      Zv = pspool.tile([P, CC], F32, tag="ps")
                      nc.tensor.matmul(Zv[:], lhsTh[l][:], v[:], start=True, stop=True)

                      s1 = wpool.tile([P, CC], SD, tag="s1")
                      nc.scalar.activation(
                          s1[:], Zp[:], AF.Derivative_silu, bias=bias_r(l + 1, 0)
                      )
                      T = wpool.tile([P, CC], SD, tag="T")
                      nc.scalar.activation(
                          T[:], Zp[:], AF.Tanh, bias=bias_r(l + 1, 1), scale=0.5
                      )
                      A = wpool.tile([P, CC], SD, tag="A")
                      nc.scalar.activation(A[:], Zu[:], AF.Square)
                      sig = wpool.tile([P, CC], SD, tag="sig")
                      nc.vector.tensor_scalar(
                          sig[:], T[:], 0.5, 0.5, ALU.mult, ALU.add
                      )
                      a = wpool.tile([P, CC], SD, tag="a")
                      nc.vector.scalar_tensor_tensor(
                          a[:], Zp[:], bias_r(l + 1, 0), sig[:], ALU.add, ALU.mult
                      )
                      G = wpool.tile([P, CC], SD, tag="G")
                      nc.vector.tensor_tensor(G[:], s1[:], T[:], ALU.mult)
                      s2p = wpool.tile([P, CC], SD, tag="s2p")
                      nc.vector.scalar_tensor_tensor(
                          s2p[:], T[:], 0.5, G[:], ALU.mult, ALU.subtract
                      )
                      u = wpool.tile([P, CC], SD, tag="u")
                      nc.vector.tensor_tensor(u[:], s1[:], Zu[:], ALU.mult)
                      gm = wpool.tile([P, CC], SD, tag="gm")
                      nc.vector.tensor_tensor(gm[:], s1[:], Zg[:], ALU.mult)
                      q = wpool.tile([P, CC], SD, tag="q")
                      nc.vector.tensor_tensor(q[:], s1[:], Zv[:], ALU.mult)
                      Bq = wpool.tile([P, CC], SD, tag="Bq")
                      nc.vector.scalar_tensor_tensor(
                          Bq[:], s2p[:], 0.5, A[:], ALU.add, ALU.mult
                      )
                      v = wpool.tile([P, CC], SD, tag="v")
                      nc.vector.tensor_tensor(v[:], Bq[:], q[:], ALU.add)

                  # final layer
                  Zf = pspool.tile([NG, CC], F32, tag="ps")
                  nc.tensor.matmul(Zf[:], lhsTf[:], a[:], start=True, stop=True)
                  Zuf = pspool.tile([NG, CC], F32, tag="ps")
                  nc.tensor.matmul(Zuf[:], lhsTf[:], u[:], start=True, stop=True)
                  Zgf = pspool.tile([NG, CC], F32, tag="ps")
                  nc.tensor.matmul(Zgf[:], lhsTf[:], gm[:], start=True, stop=True)
                  Zvf = pspool.tile([NG, CC], F32, tag="ps")
                  nc.tensor.matmul(Zvf[:], lhsTf[:], v[:], start=True, stop=True)

                  # copy [4, CC] psum chunks into the [16, C] staging buffer
                  for si, Zs in enumerate((Zf, Zuf, Zgf, Zvf)):
                      dst = zf_st[32 * si : 32 * si + 4, ci * CC : (ci + 1) * CC]
                      nc.scalar.copy(dst, Zs[:])

              run_D = run_C and (DBG_NCHUNK is None)
              vT = sgpool.tile([P, NB, 1], F32, tag="vT")
              if run_D:
                  # bridge: absorb the DMA-completion wait on this queue before
                  # the repack DMAs (DMA instructions fit only one sync wait).
                  scr2 = cpool.tile([1, 4], SD, tag="scr2")
                  nc.sync.dma_start(scr2[:, 0:1], rhs0[5 * (NG - 1) + 4 : 5 * NG, 0:1])
                  # repack staging -> sgrid layout (per stream x group to keep
                  # the DMA access patterns simple: one source partition fans
                  # out to 32 destination partitions)
                  for si in range(4):
                      for g in range(NG):
                          src = zf_st[
                              32 * si + g : 32 * si + g + 1, :
                          ].rearrange("one (q b n) -> one q b n", q=32, b=NB)
                          nc.sync.dma_start(
                              zf_sg[si][32 * g : 32 * (g + 1), :, :], src
                          )

                  # ---- stage D: phi assembly + reduction ----
                  Tf = sgpool.tile([P, NB, NSTEP], F32, tag="Tf")
                  nc.scalar.activation(Tf[:], zf_sg[0][:], AF.Tanh, bias=bfh[:], scale=0.5)
                  sp = sgpool.tile([P, NB, NSTEP], F32, tag="sp")
                  nc.vector.tensor_mul(sp[:], Tf[:], Tf[:])
                  nc.vector.tensor_scalar(sp[:], sp[:], -0.25, 0.25, ALU.mult, ALU.add)
                  S = sgpool.tile([P, NB, NSTEP], F32, tag="S")
                  nc.vector.tensor_tensor(S[:], zf_sg[2][:], zf_sg[3][:], ALU.add)
                  Q = sgpool.tile([P, NB, NSTEP], F32, tag="Q")
                  nc.vector.tensor_mul(Q[:], zf_sg[1][:], zf_sg[1][:])
                  nc.vector.tensor_mul(Q[:], Q[:], Tf[:])
                  nc.vector.tensor_tensor(S[:], S[:], Q[:], ALU.subtract)
                  nc.vector.tensor_tensor(S[:], sp[:], S[:], ALU.mult)
                  nc.vector.tensor_reduce(vT[:], S[:], mybir.AxisListType.X, ALU.add)
              else:
                  nc.vector.memset(vT[:], 0.0)

              # ---- outputs ----
              if run_D:
                  scr3 = cpool.tile([1, 4], SD, tag="scr3")
                  nc.sync.dma_start(scr3[:, 0:1], zf_sg[3][0:1, 0:1, 0:1])
              yv = out_d[:].rearrange("(b p) c -> p b c", p=P)
              nc.sync.dma_start(yv[:, :, 0:1], sfull[:, :, NSTEP : NSTEP + 1])
              nc.sync.dma_start(yv[:, :, 1:2], vT[:])

    _legalize_waits(nc)
    return nc


def _prep_host(inputs):
    rnorm = np.ascontiguousarray(np.asarray(inputs["rnorm"], dtype=np.float32))
    W0 = np.asarray(inputs["W0"], dtype=np.float32)
    b0 = np.asarray(inputs["b0"], dtype=np.float32)
    Wh = np.asarray(inputs["Wh"], dtype=np.float32)
    bh = np.asarray(inputs["bh"], dtype=np.float32)
    Wf = np.asarray(inputs["Wf"], dtype=np.float32)
    bf = np.asarray(inputs["bf"], dtype=np.float32)

    sd_np = mybir.dt.np(SD)

    # t-row pattern: col j -> t = DT * (j % NSTEP), plus a ones plane
    trow = np.ones((P, 2, C // P), np.float32)
    trow[:, 0, :] = (
        DT * np.tile(np.arange(NSTEP, dtype=np.float32), C // NSTEP)
    ).reshape(P, C // P)

    # seed lhsTs [5*NG, P]
    lhsT0 = np.zeros((5 * NG, P), np.float32)
    lhsTg = np.zeros((5 * NG, P), np.float32)
    lhsTu = np.zeros((5 * NG, P), np.float32)
    for g in range(NG):
        cols = slice(32 * g, 32 * (g + 1))
        lhsT0[5 * g + 0, cols] = W0[:, 0]          # t coefficient
        lhsT0[5 * g + 1, cols] = W0[:, 1]          # s coefficient
        lhsTg[5 * g + 2, cols] = W0[:, 1]          # Ds row
        lhsTg[5 * g + 3, cols] = W0[:, 0] * DT     # ones row -> dhdt*dt
        lhsTu[5 * g + 4, cols] = W0[:, 1] * np.sqrt(0.5) * SIG

    lhsTh = np.zeros((NH, P, P), np.float32)
    for l in range(NH):
        for g in range(NG):
            blk = slice(32 * g, 32 * (g + 1))
            lhsTh[l, blk, blk] = Wh[l].T
    lhsTf = np.zeros((P, NG), np.float32)
    for g in range(NG):
        lhsTf[32 * g : 32 * (g + 1), g] = Wf[0]

    bias = np.zeros((P, 4, 2), np.float32)
    bias[:, 0, 0] = np.tile(b0, NG)
    bias[:, 0, 1] = 0.5 * bias[:, 0, 0]
    for l in range(NH):
        bias[:, l + 1, 0] = np.tile(bh[l], NG)
        bias[:, l + 1, 1] = 0.5 * bias[:, l + 1, 0]
    bfh = np.full((P, 1), 0.5 * bf[0], np.float32)

    shared = {
        "trow": trow.astype(sd_np),
        "lhsT0": lhsT0.astype(sd_np),
        "lhsTg": lhsTg.astype(sd_np),
        "lhsTu": lhsTu.astype(sd_np),
        "lhsTh": lhsTh.astype(sd_np),
        "lhsTf": lhsTf.astype(sd_np),
        "bias": bias,
        "bfh": bfh,
    }

    in_maps = []
    for core in range(NCORE):
        shard = rnorm[core * BC : (core + 1) * BC]          # [1024, 128]
        sg = np.ascontiguousarray(
            shard.reshape(NB, P, NSTEP).transpose(1, 0, 2).reshape(P, NB * NSTEP)
        )
        in_maps.append({"rn_sg": sg, **shared})
    return in_maps


last_perf = {}


def kernel(trace=False, **inputs) -> np.ndarray:
    if "nc" not in _CACHE:
        _CACHE["nc"] = _build_program()
    nc = _CACHE["nc"]
    in_maps = _prep_host(inputs)
    res = run_bass_kernel_spmd(nc, in_maps, list(range(NCORE)), trace=trace)
    last_perf["exec_time_ns"] = res.exec_time_ns
    out = np.empty((B, 2), np.float32)
    for core in range(NCORE):
        yt = res.results[core]["yT"]                        # [1024, 2]
        out[core * BC : (core + 1) * BC] = yt
    return out



# revision 13
# speedup vs baseline: 1.3633x; 1.3633x over previous
"""Trainium2 Bass kernel for the deep-hedging Milstein SDE loss.

Math: the reference scan collapses (see derivation in comments below):
  s_{n+1} = s_n * m_n,  m_n = c0 + c1*r_n + c2*r_n^2
  v_{n+1} = v_n + sp_n * (Zw_n - Zu_n^2 * Tf_n)            [per-point phi terms]
where the per-point quantities come from a forward-mode jet of the holding
MLP with THREE streams:
  a  : primal silu chain
  u  : first-order tangent along (0, sqrt(0.5*dt)*SIG*s*r)   [2nd-order probe]
  w  : merged gamma + second-order stream:
         w0 = silu'(z)*Mg + silu''(z)*Mu^2
         w' = silu'(z)*Zw + silu''(z)*Zu^2
(the gamma direction is (dt, Ds); gamma and the 2nd-order stream propagate
with the same linear rule and are only ever used summed, so they merge.)

Layout per core (1024 paths, 128 steps):
  sgrid [128 part = p, 8 blocks b, 128 steps n], path_local = b*128 + p.
  MLP groups g = b // 2 (4 groups of 2 blocks); point column within a group:
      j = x*128 + p,   x = b2*128 + n,  b = 2*g + b2.
  Chunk ci = x in [4ci, 4ci+4) -> 512 columns.

Stage B repack is done on the PE: S5T [p, x, kgp(32: 20 real + 12 pad)]
holds the 5 value planes (t, ones, s, Ds, s*r) interleaved so that one
[128,128] PE transpose per chunk yields the matmul rhs [(x4, kg32), p].
A plain DMA cannot do this repack: the cost model charges per-partition
bytes and the BIR verifier requires the partition-crossing dim first on
both sides, which forbids partition-transposing DMAs.

Engine split per chunk (V1 CoreSim cost model):
  PE  : 1 transpose + 12 L0 sub-matmuls + 9 hidden + 3 final  (~3.3us)
  ACT : silu' + tanh per layer (+1 staging copy)              (~5.5-6.1us)
  DVE : all-SBUF f16 stt/ts ops at 4x mode (193ns each)       (~5.7us)
  Pool: PSUM-reading stt ops at flat 427ns                    (~6.1us)
"""

import os

import numpy as np

import concourse.bass as bass
import concourse.mybir as mybir
from concourse import tile
from concourse.bass_utils import run_bass_kernel_spmd


# problem constants (hardcoded per spec)
B = 8192
NSTEP = 128
NCORE = 8
BC = B // NCORE          # 1024 paths per core
P = 128                  # partitions
NB = BC // P             # 8 path blocks
WIDTH = 32
NG = 4                   # feature groups on partitions
NH = 3                   # hidden layers
NX = 2 * NSTEP           # 256 x-values (b2, n)
C = NX * P               # 32768 point-columns per group
CC = 512                 # chunk columns (4 x-values * 128 p)
NCHUNK = NX // 4         # 64
KREAL = 20               # 5 value planes * 4 groups
KG = 32                  # padded plane rows per x in S5T
T0, T1 = 0.0, 1.0
MU, SIG = 1.0, 1.0
DT = (T1 - T0) / NSTEP
SQDT = float(np.sqrt(DT))

F32 = mybir.dt.float32
AF = mybir.ActivationFunctionType
ALU = mybir.AluOpType

SD = mybir.dt.float16

_CACHE = {}
DBG_NCHUNK = int(os.environ.get("KDBG_NCHUNK", "0")) or None


def _legalize_waits(nc):
    """Split long on_wait lists into standalone single-wait NoOps.

    This walrus rejects instructions whose sync_info carries more waits
    than the ISA encoding holds. Tile emits up to one wait per logical
    processor, so spill the excess onto NoOps on the same engine queue,
    which execute in order before the real instruction.
    """
    ctr = 0
    for bb in nc.main_func.blocks:
        out = []
        for ins in bb.instructions:
            si = ins.sync_info
            if si is not None and si.on_wait:
                limit = 1
                waits = list(si.on_wait)
                if len(waits) > limit:
                    spill, keep = waits[:-limit], waits[-limit:]
                    for w in spill:
                        ctr += 1
                        nop = mybir.InstNoOp(name=f"waitnop_{ctr}", ins=[], outs=[])
                        nop.engine = ins.engine
                        nop.sync_info = mybir.SyncInfo(on_wait=[w], on_update=[])
                        out.append(nop)
                    si.on_wait = keep
            out.append(ins)
        bb.instructions = out


def _build_program():
    nc = bass.Bass()

    rn_d = nc.declare_dram_parameter("rn_sg", [P, NB * NSTEP], F32, isOutput=False)
    tk_d = nc.declare_dram_parameter("tk", [P, NX * 8], SD, isOutput=False)
    id_d = nc.declare_dram_parameter("ident", [P, P], SD, isOutput=False)
    lhsTL_d = nc.declare_dram_parameter("lhsTL", [12, P, P], SD, isOutput=False)
    lhsTh_d = nc.declare_dram_parameter("lhsTh", [NH, P, P], SD, isOutput=False)
    lhsTf_d = nc.declare_dram_parameter("lhsTf", [P, NG], SD, isOutput=False)
    bias_d = nc.declare_dram_parameter("bias", [P, 4, 2], F32, isOutput=False)
    bfh_d = nc.declare_dram_parameter("bfh", [P, 1], F32, isOutput=False)
    out_d = nc.declare_dram_parameter("yT", [BC, 2], F32, isOutput=True)

    # m_n = c0 + c1*r + c2*r^2
    c0 = 1.0 + MU * DT - 0.5 * SIG * SIG * DT
    c1 = SIG * SQDT
    c2 = 0.5 * SIG * SIG * DT

    with tile.TileContext(nc) as tc:
        with (
            tc.tile_pool(name="const", bufs=1) as cpool,
            tc.tile_pool(name="sg", bufs=1) as sgpool,
            tc.tile_pool(name="work", bufs=3) as wpool,
            tc.tile_pool(name="psum", bufs=7, space="PSUM") as pspool,
            tc.tile_pool(name="pst", bufs=1, space="PSUM") as pstpool,
        ):
            # ---- constants ----
            ident = cpool.tile([P, P], SD, tag="ident")
            lhsTL = [
                cpool.tile([P, P], SD, tag=f"lhsTL{i}", name=f"lhsTL{i}")
                for i in range(12)
            ]
            lhsTh = [
                cpool.tile([P, P], SD, tag=f"lhsTh{l}", name=f"lhsTh{l}")
                for l in range(NH)
            ]
            lhsTf = cpool.tile([P, NG], SD, tag="lhsTf")
            bias = cpool.tile([P, 4, 2], F32, tag="bias")
            bfh = cpool.tile([P, 1], F32, tag="bfh")
            nc.sync.dma_start(ident[:], id_d[:])
            for i in range(12):
                nc.sync.dma_start(lhsTL[i][:], lhsTL_d[i])
            for l in range(NH):
                nc.sync.dma_start(lhsTh[l][:], lhsTh_d[l])
            nc.sync.dma_start(lhsTf[:], lhsTf_d[:])
            nc.sync.dma_start(bias[:], bias_d[:])
            nc.sync.dma_start(bfh[:], bfh_d[:])

            def bias_r(l, h):
                return bias[:, l, h : h + 1]

            # ---- stage A: sgrid GBM math -> S5T staging ----
            # S5T[p, x, kgp]: kgp = 4k+g; planes k: 0 t, 1 ones, 2 s, 3 Ds, 4 s*r
            S5T = sgpool.tile([P, NX, KG], SD, tag="S5T")
            # constant planes (t, ones) from DRAM
            nc.sync.dma_start(
                S5T[:, :, 0:8],
                tk_d[:].rearrange("p (x k) -> p x k", k=8),
            )
            # pad rows: keep finite for the transpose passthrough
            nc.gpsimd.memset(S5T[:, :, KREAL:KG], 0.0)

            rs = sgpool.tile([P, NB, NSTEP], F32, tag="rs")
            nc.sync.dma_start(rs[:], rn_d[:].rearrange("p (b n) -> p b n", b=NB))
            scr = sgpool.tile([P, NB, NSTEP], F32, tag="scr")
            m = sgpool.tile([P, NB, NSTEP], F32, tag="m")
            # m = (c2*r + c1)*r + c0
            nc.vector.tensor_scalar(scr[:], rs[:], c2, c1, ALU.mult, ALU.add)
            nc.vector.scalar_tensor_tensor(m[:], scr[:], 0.0, rs[:], ALU.add, ALU.mult)
            nc.vector.tensor_scalar(m[:], m[:], 1.0, c0, ALU.mult, ALU.add)

            sfull = sgpool.tile([P, NB, NSTEP + 1], F32, tag="sfull")
            nc.vector.memset(sfull[:, :, 0:1], 1.0)
            for b in range(NB):
                nc.vector.tensor_tensor_scan(
                    sfull[:, b, 1 : NSTEP + 1],
                    m[:, b, :],
                    m[:, b, :],
                    1.0,
                    ALU.mult,
                    ALU.bypass,
                )
            sN = sfull[:, :, 0:NSTEP]

            # plane views into S5T: iteration (p, g, b2, n) matching sgrid (p, b=2g+b2, n)
            def plane(k):
                return S5T[:].rearrange("p (b2 n) (k g) -> k p g b2 n", k=8, b2=2)[k]

            def sg_gb(t_ap):
                # sgrid [p, b, n] -> [p, g, b2, n]
                return t_ap.rearrange("p (g b2) n -> p g b2 n", g=NG)

            # s plane (Pool), Ds plane (DVE), s*r plane (Pool)
            nc.gpsimd.tensor_copy(plane(2), sg_gb(sN))
            nc.vector.scalar_tensor_tensor(
                plane(3), sg_gb(m[:]), 1.0, sg_gb(sN), ALU.subtract, ALU.mult
            )
            nc.gpsimd.tensor_tensor(plane(4), sg_gb(sN), sg_gb(rs[:]), ALU.mult)

            # ---- staging for stage D: rows 32s+g, cols j = x*128+p ----
            staging = sgpool.tile([P, C], SD, tag="staging")

            # ---- stage C: chunked MLP jet ----
            nchunk = DBG_NCHUNK or NCHUNK
            for ci in range(nchunk):
                # PE repack: [128 p, 4x*32kg] -> [4x*32kg, 128 p]
                pst = pstpool.tile([P, P], SD, tag="pst")
                nc.tensor.transpose(pst[:], S5T[:, 4 * ci : 4 * ci + 4, :], ident[:])
                rhsb = wpool.tile([P, P], SD, tag="rhsb")
                nc.vector.tensor_copy(rhsb[:], pst[:])

                Z0 = pspool.tile([P, CC], F32, tag="ps")
                Mg = pspool.tile([P, CC], F32, tag="ps")
                Mu = pspool.tile([P, CC], F32, tag="ps")
                for xi in range(4):
                    sl = slice(xi * P, (xi + 1) * P)
                    rv = rhsb[:]
                    nc.tensor.matmul(Z0[:, sl], lhsTL[0 + xi][:], rv, start=True, stop=True)
                    nc.tensor.matmul(Mg[:, sl], lhsTL[4 + xi][:], rv, start=True, stop=True)
                    nc.tensor.matmul(Mu[:, sl], lhsTL[8 + xi][:], rv, start=True, stop=True)

                a = u = w = None
                Zp, Zu, Zw = Z0, Mu, Mg
                for l in range(4):
                    if l > 0:
                        Zp = pspool.tile([P, CC], F32, tag="ps")
                        Zu = pspool.tile([P, CC], F32, tag="ps")
                        Zw = pspool.tile([P, CC], F32, tag="ps")
                        nc.tensor.matmul(Zp[:], lhsTh[l - 1][:], a[:], start=True, stop=True)
                        nc.tensor.matmul(Zu[:], lhsTh[l - 1][:], u[:], start=True, stop=True)
                        nc.tensor.matmul(Zw[:], lhsTh[l - 1][:], w[:], start=True, stop=True)

                    s1 = wpool.tile([P, CC], SD, tag="s1")
                    nc.scalar.activation(
                        s1[:], Zp[:], AF.Derivative_silu, bias=bias_r(l, 0)
                    )
                    T = wpool.tile([P, CC], SD, tag="T")
                    nc.scalar.activation(
                        T[:], Zp[:], AF.Tanh, bias=bias_r(l, 1), scale=0.5
                    )
                    # zu16 = f16 copy of Zu: unlocks u/A as cheap f16 ops
                    zu16 = wpool.tile([P, CC], SD, tag="zu16")
                    nc.scalar.activation(zu16[:], Zu[:], AF.Identity)
                    # sigma = 0.5*T + 0.5                        (Pool f16)
                    sg = wpool.tile([P, CC], SD, tag="sg")
                    nc.gpsimd.tensor_scalar(sg[:], T[:], 0.5, 0.5, ALU.mult, ALU.add)
                    # a' = (Zp + b) * sigma                      (DVE, PSUM)
                    a = wpool.tile([P, CC], SD, tag="a")
                    nc.vector.scalar_tensor_tensor(
                        a[:], Zp[:], bias_r(l, 0), sg[:], ALU.add, ALU.mult
                    )
                    # u' = zu16 * s1                             (Pool f16)
                    un = wpool.tile([P, CC], SD, tag="u")
                    nc.gpsimd.tensor_tensor(un[:], zu16[:], s1[:], ALU.mult)
                    # q = Zw * s1                                (DVE, PSUM)
                    q = wpool.tile([P, CC], SD, tag="q")
                    nc.vector.scalar_tensor_tensor(
                        q[:], Zw[:], 0.0, s1[:], ALU.add, ALU.mult
                    )
                    # A = zu16^2                                 (Pool f16)
                    A = wpool.tile([P, CC], SD, tag="A")
                    nc.gpsimd.tensor_tensor(A[:], zu16[:], zu16[:], ALU.mult)
                    # e2 = (s1 - 0.5)*T ; phi' = (e2 - 0.5)*A = -silu''*A   (DVE 4x)
                    e2 = wpool.tile([P, CC], SD, tag="e2")
                    nc.vector.scalar_tensor_tensor(
                        e2[:], s1[:], 0.5, T[:], ALU.subtract, ALU.mult
                    )
                    ph = wpool.tile([P, CC], SD, tag="ph")
                    nc.vector.scalar_tensor_tensor(
                        ph[:], e2[:], 0.5, A[:], ALU.subtract, ALU.mult
                    )
                    # w' = q - phi'                              (Pool f16)
                    w = wpool.tile([P, CC], SD, tag="w")
                    nc.gpsimd.tensor_tensor(w[:], q[:], ph[:], ALU.subtract)
                    u = un

                # final layer: three [4, CC] matmuls into ONE psum bank at
                # partition offsets 0/32/64, then a single copy to staging
                F = pspool.tile([P, CC], F32, tag="ps")
                nc.tensor.matmul(F[0:4, :], lhsTf[:], a[:], start=True, stop=True)
                nc.tensor.matmul(F[32:36, :], lhsTf[:], u[:], start=True, stop=True)
                nc.tensor.matmul(F[64:68, :], lhsTf[:], w[:], start=True, stop=True)
                # staging is p-major (col j = p*NX + x) so the stage-D
                # repack DMA gets a contiguous last dim; scatter the chunk's
                # (xi, p) columns accordingly
                dst = staging[0:68, :].rearrange("r (p x) -> r p x", p=P)[
                    :, :, 4 * ci : 4 * ci + 4
                ]
                fin = F[0:68, :].rearrange("r (xi p) -> r p xi", xi=4)
                if ci & 1:
                    nc.vector.tensor_copy(dst, fin)
                else:
                    nc.scalar.copy(dst, fin)

            # ---- stage D: repack + phi assembly + reduction ----
            zf_sg = sgpool.tile([P, NB, NSTEP], SD, tag="zf_sg")
            uf_sg = sgpool.tile([P, NB, NSTEP], SD, tag="uf_sg")
            wf_sg = sgpool.tile([P, NB, NSTEP], SD, tag="wf_sg")
            if DBG_NCHUNK is None:
                for s, sgt in enumerate((zf_sg, uf_sg, wf_sg)):
                    for g in range(NG):
                        row = 32 * s + g
                        src = staging[row : row + 1, :].rearrange(
                            "one (p x) -> one p x", p=P
                        )
                        dst = sgt[:, 2 * g : 2 * g + 2, :].rearrange(
                            "p b2 n -> p (b2 n)"
                        )
                        nc.sync.dma_start(dst, src)

                Tf = sgpool.tile([P, NB, NSTEP], SD, tag="Tf")
                nc.scalar.activation(Tf[:], zf_sg[:], AF.Tanh, bias=bfh[:], scale=0.5)
                U2 = sgpool.tile([P, NB, NSTEP], SD, tag="U2")
                nc.vector.scalar_tensor_tensor(
                    U2[:], uf_sg[:], 0.0, uf_sg[:], ALU.add, ALU.mult
                )
                Q = sgpool.tile([P, NB, NSTEP], SD, tag="Q")
                nc.vector.scalar_tensor_tensor(
                    Q[:], U2[:], 0.0, Tf[:], ALU.add, ALU.mult
                )
                Dd = sgpool.tile([P, NB, NSTEP], SD, tag="Dd")
                nc.vector.scalar_tensor_tensor(
                    Dd[:], wf_sg[:], 0.0, Q[:], ALU.add, ALU.subtract
                )
                T2 = sgpool.tile([P, NB, NSTEP], SD, tag="T2")
                nc.vector.scalar_tensor_tensor(
                    T2[:], Tf[:], 0.0, Tf[:], ALU.add, ALU.mult
                )
                sp = sgpool.tile([P, NB, NSTEP], SD, tag="sp")
                nc.vector.tensor_scalar(sp[:], T2[:], -0.25, 0.25, ALU.mult, ALU.add)
                Sd = sgpool.tile([P, NB, NSTEP], SD, tag="Sd")
                vT = sgpool.tile([P, NB], F32, tag="vT")
                for b in range(NB):
                    nc.vector.scalar_tensor_tensor(
                        Sd[:, b, :],
                        Dd[:, b, :],
                        0.0,
                        sp[:, b, :],
                        ALU.add,
                        ALU.mult,
                        accum_out=vT[:, b : b + 1],
                    )
            else:
                vT = sgpool.tile([P, NB], F32, tag="vT")
                nc.vector.memset(vT[:], 0.0)

            # ---- outputs ----
            yv = out_d[:].rearrange("(b p) c -> p b c", p=P)
            nc.sync.dma_start(yv[:, :, 0:1], sfull[:, :, NSTEP : NSTEP + 1])
            nc.sync.dma_start(
                yv[:, :, 1:2], vT[:].rearrange("p (b one) -> p b one", one=1)
            )

    _legalize_waits(nc)
    return nc


def _prep_host(inputs):
    rnorm = np.ascontiguousarray(np.asarray(inputs["rnorm"], dtype=np.float32))
    W0 = np.asarray(inputs["W0"], dtype=np.float32)
    b0 = np.asarray(inputs["b0"], dtype=np.float32)
    Wh = np.asarray(inputs["Wh"], dtype=np.float32)
    bh = np.asarray(inputs["bh"], dtype=np.float32)
    Wf = np.asarray(inputs["Wf"], dtype=np.float32)
    bf = np.asarray(inputs["bf"], dtype=np.float32)

    sd_np = mybir.dt.np(SD)

    # tk[p, x*8 + kgp]: t/ones planes; t = DT*(x mod 128) for every p
    tk = np.zeros((P, NX, 8), np.float32)
    n_of_x = np.tile(np.arange(NSTEP, dtype=np.float32), 2)
    for g in range(NG):
        tk[:, :, 0 + g] = (DT * n_of_x)[None, :]
        tk[:, :, 4 + g] = 1.0

    ident = np.eye(P, dtype=np.float32)

    # lhsT rows r = 4k+g; planes k: 0 t, 1 ones, 2 s, 3 Ds, 4 s*r
    # K=128 with 4 quadrant-masked variants per stream: variant xi has the
    # 32-row coef block at partitions 32*xi and zeros elsewhere
    l0 = np.zeros((KG, P), np.float32)
    lg = np.zeros((KG, P), np.float32)
    lu = np.zeros((KG, P), np.float32)
    for g in range(NG):
        cols = slice(32 * g, 32 * (g + 1))
        l0[4 * 0 + g, cols] = W0[:, 0]                          # t
        l0[4 * 2 + g, cols] = W0[:, 1]                          # s
        lg[4 * 1 + g, cols] = W0[:, 0] * DT                     # ones -> dhdt*dt
        lg[4 * 3 + g, cols] = W0[:, 1]                          # Ds
        lu[4 * 4 + g, cols] = W0[:, 1] * SIG * float(np.sqrt(0.5 * DT))
    lhsTL = np.zeros((12, P, P), np.float32)
    for s, blk in enumerate((l0, lg, lu)):
        for xi in range(4):
            lhsTL[s * 4 + xi, 32 * xi : 32 * (xi + 1), :] = blk

    lhsTh = np.zeros((NH, P, P), np.float32)
    for l in range(NH):
        for g in range(NG):
            blk = slice(32 * g, 32 * (g + 1))
            lhsTh[l, blk, blk] = Wh[l].T
    lhsTf = np.zeros((P, NG), np.float32)
    for g in range(NG):
        lhsTf[32 * g : 32 * (g + 1), g] = Wf[0]

    bias = np.zeros((P, 4, 2), np.float32)
    bias[:, 0, 0] = np.tile(b0, NG)
    bias[:, 0, 1] = 0.5 * bias[:, 0, 0]
    for l in range(NH):
        bias[:, l + 1, 0] = np.tile(bh[l], NG)
        bias[:, l + 1, 1] = 0.5 * bias[:, l + 1, 0]
    bfh = np.full((P, 1), 0.5 * bf[0], np.float32)

    shared = {
        "tk": tk.reshape(P, NX * 8).astype(sd_np),
        "ident": ident.astype(sd_np),
        "lhsTL": lhsTL.astype(sd_np),
        "lhsTh": lhsTh.astype(sd_np),
        "lhsTf": lhsTf.astype(sd_np),
        "bias": bias,
        "bfh": bfh,
    }

    in_maps = []
    for core in range(NCORE):
        shard = rnorm[core * BC : (core + 1) * BC]          # [1024, 128]
        sg = np.ascontiguousarray(
            shard.reshape(NB, P, NSTEP).transpose(1, 0, 2).reshape(P, NB * NSTEP)
        )
        in_maps.append({"rn_sg": sg, **shared})
    return in_maps


last_perf = {}


def kernel(trace=False, **inputs) -> np.ndarray:
    if "nc" not in _CACHE:
        _CACHE["nc"] = _build_program()
    nc = _CACHE["nc"]
    in_maps = _prep_host(inputs)
    res = run_bass_kernel_spmd(nc, in_maps, list(range(NCORE)), trace=trace)
    last_perf["exec_time_ns"] = res.exec_time_ns
    out = np.empty((B, 2), np.float32)
    for core in range(NCORE):
        yt = res.results[core]["yT"]                        # [1024, 2]
        out[core * BC : (core + 1) * BC] = yt
    return out


# revision 14
# speedup vs baseline: 3.1437x; 2.3059x over previous
"""Trainium2 Bass kernel for the deep-hedging Milstein SDE loss.

Math: the reference scan collapses (see derivation in comments below):
  s_{n+1} = s_n * m_n,  m_n = c0 + c1*r_n + c2*r_n^2
  v_{n+1} = v_n + sp_n * (Zw_n - Zu_n^2 * Tf_n)            [per-point phi terms]
where the per-point quantities come from a forward-mode jet of the holding
MLP with THREE streams:
  a  : primal silu chain
  u  : first-order tangent along (0, sqrt(0.5*dt)*SIG*s*r)   [2nd-order probe]
  w  : merged gamma + second-order stream:
         w0 = silu'(z)*Mg + silu''(z)*Mu^2
         w' = silu'(z)*Zw + silu''(z)*Zu^2
(the gamma direction is (dt, Ds); gamma and the 2nd-order stream propagate
with the same linear rule and are only ever used summed, so they merge.)

Layout per core (1024 paths, 128 steps):
  sgrid [128 part = p, 8 blocks b, 128 steps n], path_local = b*128 + p.
  MLP groups g = b // 2 (4 groups of 2 blocks); point column within a group:
      j = x*128 + p,   x = b2*128 + n,  b = 2*g + b2.
  Chunk ci = x in [4ci, 4ci+4) -> 512 columns.

Stage B repack is done on the PE: S5T [p, x, kgp(32: 20 real + 12 pad)]
holds the 5 value planes (t, ones, s, Ds, s*r) interleaved so that one
[128,128] PE transpose per chunk yields the matmul rhs [(x4, kg32), p].
A plain DMA cannot do this repack: the cost model charges per-partition
bytes and the BIR verifier requires the partition-crossing dim first on
both sides, which forbids partition-transposing DMAs.

Engine split per chunk (V1 CoreSim cost model):
  PE  : 1 transpose + 12 L0 sub-matmuls + 9 hidden + 3 final  (~3.3us)
  ACT : silu' + tanh per layer (+1 staging copy)              (~5.5-6.1us)
  DVE : all-SBUF f16 stt/ts ops at 4x mode (193ns each)       (~5.7us)
  Pool: PSUM-reading stt ops at flat 427ns                    (~6.1us)
"""

import os

import numpy as np

import concourse.bass as bass
import concourse.mybir as mybir
from concourse import tile
from concourse.bass_utils import run_bass_kernel_spmd


# problem constants (hardcoded per spec)
B = 8192
NSTEP = 128
NCORE = 8
BC = B // NCORE          # 1024 paths per core
P = 128                  # partitions
NB = BC // P             # 8 path blocks
WIDTH = 32
NG = 4                   # feature groups on partitions
NH = 3                   # hidden layers
NX = 2 * NSTEP           # 256 x-values (b2, n)
C = NX * P               # 32768 point-columns per group
CC = 512                 # chunk columns (4 x-values * 128 p)
NCHUNK = NX // 4         # 64
KREAL = 20               # 5 value planes * 4 groups
KG = 32                  # padded plane rows per x in S5T
T0, T1 = 0.0, 1.0
MU, SIG = 1.0, 1.0
DT = (T1 - T0) / NSTEP
SQDT = float(np.sqrt(DT))

F32 = mybir.dt.float32
AF = mybir.ActivationFunctionType
ALU = mybir.AluOpType

SD = mybir.dt.float16

_CACHE = {}
DBG_NCHUNK = int(os.environ.get("KDBG_NCHUNK", "0")) or None


def _legalize_waits(nc):
    """Split long on_wait lists into standalone single-wait NoOps.

    This walrus rejects instructions whose sync_info carries more waits
    than the ISA encoding holds. Tile emits up to one wait per logical
    processor, so spill the excess onto NoOps on the same engine queue,
    which execute in order before the real instruction.
    """
    ctr = 0
    for bb in nc.main_func.blocks:
        out = []
        for ins in bb.instructions:
            si = ins.sync_info
            if si is not None and si.on_wait:
                limit = 1
                waits = list(si.on_wait)
                if len(waits) > limit:
                    spill, keep = waits[:-limit], waits[-limit:]
                    for w in spill:
                        ctr += 1
                        nop = mybir.InstNoOp(name=f"waitnop_{ctr}", ins=[], outs=[])
                        nop.engine = ins.engine
                        nop.sync_info = mybir.SyncInfo(on_wait=[w], on_update=[])
                        out.append(nop)
                    si.on_wait = keep
            out.append(ins)
        bb.instructions = out


def _build_program():
    nc = bass.Bass()

    rn_d = nc.declare_dram_parameter("rn_sg", [P, NB * NSTEP], F32, isOutput=False)
    tk_d = nc.declare_dram_parameter("tk", [P, NX * 8], SD, isOutput=False)
    id_d = nc.declare_dram_parameter("ident", [P, P], SD, isOutput=False)
    lhsTL_d = nc.declare_dram_parameter("lhsTL", [12, P, P], SD, isOutput=False)
    lhsTh_d = nc.declare_dram_parameter("lhsTh", [NH, P, P], SD, isOutput=False)
    lhsTf_d = nc.declare_dram_parameter("lhsTf", [P, NG], SD, isOutput=False)
    bias_d = nc.declare_dram_parameter("bias", [P, 4, 2], F32, isOutput=False)
    bfh_d = nc.declare_dram_parameter("bfh", [P, 1], F32, isOutput=False)
    out_d = nc.declare_dram_parameter("yT", [BC, 2], F32, isOutput=True)

    # m_n = c0 + c1*r + c2*r^2
    c0 = 1.0 + MU * DT - 0.5 * SIG * SIG * DT
    c1 = SIG * SQDT
    c2 = 0.5 * SIG * SIG * DT

    with tile.TileContext(nc) as tc:
        with (
            tc.tile_pool(name="const", bufs=1) as cpool,
            tc.tile_pool(name="sg", bufs=1) as sgpool,
            tc.tile_pool(name="work", bufs=6) as wpool,
            tc.tile_pool(name="stream", bufs=10) as spool,
            tc.tile_pool(name="psum", bufs=7, space="PSUM") as pspool,
            tc.tile_pool(name="pst", bufs=1, space="PSUM") as pstpool,
        ):
            # ---- constants ----
            ident = cpool.tile([P, P], SD, tag="ident")
            lhsTL = [
                cpool.tile([P, P], SD, tag=f"lhsTL{i}", name=f"lhsTL{i}")
                for i in range(12)
            ]
            lhsTh = [
                cpool.tile([P, P], SD, tag=f"lhsTh{l}", name=f"lhsTh{l}")
                for l in range(NH)
            ]
            lhsTf = cpool.tile([P, NG], SD, tag="lhsTf")
            bias = cpool.tile([P, 4, 2], F32, tag="bias")
            bfh = cpool.tile([P, 1], F32, tag="bfh")
            nc.sync.dma_start(ident[:], id_d[:])
            for i in range(12):
                nc.sync.dma_start(lhsTL[i][:], lhsTL_d[i])
            for l in range(NH):
                nc.sync.dma_start(lhsTh[l][:], lhsTh_d[l])
            nc.sync.dma_start(lhsTf[:], lhsTf_d[:])
            nc.sync.dma_start(bias[:], bias_d[:])
            nc.sync.dma_start(bfh[:], bfh_d[:])

            def bias_r(l, h):
                return bias[:, l, h : h + 1]

            # ---- stage A: sgrid GBM math -> S5T staging ----
            # S5T[p, x, kgp]: kgp = 4k+g; planes k: 0 t, 1 ones, 2 s, 3 Ds, 4 s*r
            S5T = sgpool.tile([P, NX, KG], SD, tag="S5T")
            # constant planes (t, ones) from DRAM
            nc.sync.dma_start(
                S5T[:, :, 0:8],
                tk_d[:].rearrange("p (x k) -> p x k", k=8),
            )
            # pad rows: keep finite for the transpose passthrough
            nc.gpsimd.memset(S5T[:, :, KREAL:KG], 0.0)

            rs = sgpool.tile([P, NB, NSTEP], F32, tag="rs")
            nc.sync.dma_start(rs[:], rn_d[:].rearrange("p (b n) -> p b n", b=NB))
            scr = sgpool.tile([P, NB, NSTEP], F32, tag="scr")
            m = sgpool.tile([P, NB, NSTEP], F32, tag="m")
            # m = (c2*r + c1)*r + c0
            nc.vector.tensor_scalar(scr[:], rs[:], c2, c1, ALU.mult, ALU.add)
            nc.vector.scalar_tensor_tensor(m[:], scr[:], 0.0, rs[:], ALU.add, ALU.mult)
            nc.vector.tensor_scalar(m[:], m[:], 1.0, c0, ALU.mult, ALU.add)

            sfull = sgpool.tile([P, NB, NSTEP + 1], F32, tag="sfull")
            nc.vector.memset(sfull[:, :, 0:1], 1.0)
            for b in range(NB):
                nc.vector.tensor_tensor_scan(
                    sfull[:, b, 1 : NSTEP + 1],
                    m[:, b, :],
                    m[:, b, :],
                    1.0,
                    ALU.mult,
                    ALU.bypass,
                )
            sN = sfull[:, :, 0:NSTEP]

            # plane views into S5T: iteration (p, g, b2, n) matching sgrid (p, b=2g+b2, n)
            def plane(k):
                return S5T[:].rearrange("p (b2 n) (k g) -> k p g b2 n", k=8, b2=2)[k]

            def sg_gb(t_ap):
                # sgrid [p, b, n] -> [p, g, b2, n]
                return t_ap.rearrange("p (g b2) n -> p g b2 n", g=NG)

            # s plane (Pool), Ds plane (DVE), s*r plane (Pool)
            nc.gpsimd.tensor_copy(plane(2), sg_gb(sN))
            nc.vector.scalar_tensor_tensor(
                plane(3), sg_gb(m[:]), 1.0, sg_gb(sN), ALU.subtract, ALU.mult
            )
            nc.gpsimd.tensor_tensor(plane(4), sg_gb(sN), sg_gb(rs[:]), ALU.mult)

            # ---- staging for stage D: rows 32s+g, cols j = x*128+p ----
            staging = sgpool.tile([P, C], SD, tag="staging")

            # ---- stage C: chunked MLP jet (wavefront-pipelined emission) ----
            # Stages per chunk c:
            #   j=0: PE transpose + DVE rhsb copy
            #   j=1..4: layer l=j-1: PE matmuls + ACT (s1, T, zu16) + elementwise
            #   j=5: final matmuls + staging copy
            # Emitting stage j of chunk k-j at iteration k keeps every engine
            # queue filled with ~6 different chunks' ready work (in-order
            # engine queues would otherwise stall on the intra-chunk chain).
            nchunk = DBG_NCHUNK or NCHUNK
            cstate = {}

            def st_transpose(c):
                pst = pstpool.tile([P, P], SD, tag="pst")
                nc.tensor.transpose(pst[:], S5T[:, 4 * c : 4 * c + 4, :], ident[:])
                rhsb = spool.tile([P, P], SD, tag="rhsb")
                nc.vector.tensor_copy(rhsb[:], pst[:])
                cstate[c] = {"rhsb": rhsb}

            def st_layer(c, l):
                S = cstate[c]
                if l == 0:
                    Zp = pspool.tile([P, CC], F32, tag="ps")
                    Zw = pspool.tile([P, CC], F32, tag="ps")
                    Zu = pspool.tile([P, CC], F32, tag="ps")
                    rv = S.pop("rhsb")
                    for xi in range(4):
                        sl = slice(xi * P, (xi + 1) * P)
                        nc.tensor.matmul(Zp[:, sl], lhsTL[0 + xi][:], rv[:], start=True, stop=True)
                        nc.tensor.matmul(Zw[:, sl], lhsTL[4 + xi][:], rv[:], start=True, stop=True)
                        nc.tensor.matmul(Zu[:, sl], lhsTL[8 + xi][:], rv[:], start=True, stop=True)
                else:
                    a_p, u_p, w_p = S.pop("a"), S.pop("u"), S.pop("w")
                    Zp = pspool.tile([P, CC], F32, tag="ps")
                    Zu = pspool.tile([P, CC], F32, tag="ps")
                    Zw = pspool.tile([P, CC], F32, tag="ps")
                    nc.tensor.matmul(Zp[:], lhsTh[l - 1][:], a_p[:], start=True, stop=True)
                    nc.tensor.matmul(Zu[:], lhsTh[l - 1][:], u_p[:], start=True, stop=True)
                    nc.tensor.matmul(Zw[:], lhsTh[l - 1][:], w_p[:], start=True, stop=True)

                s1 = wpool.tile([P, CC], SD, tag="s1")
                nc.scalar.activation(
                    s1[:], Zp[:], AF.Derivative_silu, bias=bias_r(l, 0)
                )
                T = wpool.tile([P, CC], SD, tag="T")
                nc.scalar.activation(
                    T[:], Zp[:], AF.Tanh, bias=bias_r(l, 1), scale=0.5
                )
                zu16 = wpool.tile([P, CC], SD, tag="zu16")
                nc.scalar.activation(zu16[:], Zu[:], AF.Identity)

                # sigma = 0.5*T + 0.5                        (DVE ts 4x)
                sg = wpool.tile([P, CC], SD, tag="sg")
                nc.vector.tensor_scalar(sg[:], T[:], 0.5, 0.5, ALU.mult, ALU.add)
                # a' = (Zp + b) * sigma                      (DVE stt, PSUM)
                a = spool.tile([P, CC], SD, tag="a")
                nc.vector.scalar_tensor_tensor(
                    a[:], Zp[:], bias_r(l, 0), sg[:], ALU.add, ALU.mult
                )
                # q = Zw * s1                                (DVE tt, PSUM)
                q = wpool.tile([P, CC], SD, tag="q")
                nc.vector.tensor_tensor(q[:], Zw[:], s1[:], ALU.mult)
                # u' = zu16 * s1                             (Pool)
                u = spool.tile([P, CC], SD, tag="u")
                nc.gpsimd.tensor_tensor(u[:], zu16[:], s1[:], ALU.mult)
                # A = zu16^2                                 (Pool)
                A = wpool.tile([P, CC], SD, tag="A")
                nc.gpsimd.tensor_tensor(A[:], zu16[:], zu16[:], ALU.mult)
                # P1 = s1*T; D = P1 - sigma = -silu''       (DVE tt f16 2x)
                P1 = wpool.tile([P, CC], SD, tag="P1")
                nc.vector.tensor_tensor(P1[:], s1[:], T[:], ALU.mult)
                D = wpool.tile([P, CC], SD, tag="D")
                deng = nc.vector if l == 0 else nc.gpsimd
                deng.tensor_tensor(D[:], P1[:], sg[:], ALU.subtract)
                # phi = D*A = -silu''*A                      (Pool)
                ph = wpool.tile([P, CC], SD, tag="ph")
                nc.gpsimd.tensor_tensor(ph[:], D[:], A[:], ALU.mult)
                # w' = q - phi = q + silu''*A                (Pool)
                w = spool.tile([P, CC], SD, tag="w")
                nc.gpsimd.tensor_tensor(w[:], q[:], ph[:], ALU.subtract)
                S["a"], S["u"], S["w"] = a, u, w

            def st_final(c):
                S = cstate.pop(c)
                a, u, w = S["a"], S["u"], S["w"]
                F = pspool.tile([P, CC], F32, tag="ps")
                nc.tensor.matmul(F[0:4, :], lhsTf[:], a[:], start=True, stop=True)
                nc.tensor.matmul(F[32:36, :], lhsTf[:], u[:], start=True, stop=True)
                nc.tensor.matmul(F[64:68, :], lhsTf[:], w[:], start=True, stop=True)
                dst = staging[0:68, :].rearrange("r (p x) -> r p x", p=P)[
                    :, :, 4 * c : 4 * c + 4
                ]
                fin = F[0:68, :].rearrange("r (xi p) -> r p xi", xi=4)
                if c & 1:
                    nc.vector.tensor_copy(dst, fin)
                else:
                    nc.scalar.copy(dst, fin)

            for k in range(nchunk + 6):
                for j in range(6):
                    c = k - j
                    if not (0 <= c < nchunk):
                        continue
                    if j == 0:
                        st_transpose(c)
                    elif j < 5:
                        st_layer(c, j - 1)
                    else:
                        st_final(c)

            # ---- stage D: repack + phi assembly + reduction ----
            zf_sg = sgpool.tile([P, NB, NSTEP], SD, tag="zf_sg")
            uf_sg = sgpool.tile([P, NB, NSTEP], SD, tag="uf_sg")
            wf_sg = sgpool.tile([P, NB, NSTEP], SD, tag="wf_sg")
            if DBG_NCHUNK is None:
                for s, sgt in enumerate((zf_sg, uf_sg, wf_sg)):
                    for g in range(NG):
                        row = 32 * s + g
                        src = staging[row : row + 1, :].rearrange(
                            "one (p x) -> one p x", p=P
                        )
                        dst = sgt[:, 2 * g : 2 * g + 2, :].rearrange(
                            "p b2 n -> p (b2 n)"
                        )
                        nc.sync.dma_start(dst, src)

                Tf = sgpool.tile([P, NB, NSTEP], SD, tag="Tf")
                nc.scalar.activation(Tf[:], zf_sg[:], AF.Tanh, bias=bfh[:], scale=0.5)
                U2 = sgpool.tile([P, NB, NSTEP], SD, tag="U2")
                nc.vector.scalar_tensor_tensor(
                    U2[:], uf_sg[:], 0.0, uf_sg[:], ALU.add, ALU.mult
                )
                Q = sgpool.tile([P, NB, NSTEP], SD, tag="Q")
                nc.vector.scalar_tensor_tensor(
                    Q[:], U2[:], 0.0, Tf[:], ALU.add, ALU.mult
                )
                Dd = sgpool.tile([P, NB, NSTEP], SD, tag="Dd")
                nc.vector.scalar_tensor_tensor(
                    Dd[:], wf_sg[:], 0.0, Q[:], ALU.add, ALU.subtract
                )
                T2 = sgpool.tile([P, NB, NSTEP], SD, tag="T2")
                nc.vector.scalar_tensor_tensor(
                    T2[:], Tf[:], 0.0, Tf[:], ALU.add, ALU.mult
                )
                sp = sgpool.tile([P, NB, NSTEP], SD, tag="sp")
                nc.vector.tensor_scalar(sp[:], T2[:], -0.25, 0.25, ALU.mult, ALU.add)
                Sd = sgpool.tile([P, NB, NSTEP], SD, tag="Sd")
                vT = sgpool.tile([P, NB], F32, tag="vT")
                for b in range(NB):
                    nc.vector.scalar_tensor_tensor(
                        Sd[:, b, :],
                        Dd[:, b, :],
                        0.0,
                        sp[:, b, :],
                        ALU.add,
                        ALU.mult,
                        accum_out=vT[:, b : b + 1],
                    )
            else:
                vT = sgpool.tile([P, NB], F32, tag="vT")
                nc.vector.memset(vT[:], 0.0)

            # ---- outputs ----
            yv = out_d[:].rearrange("(b p) c -> p b c", p=P)
            nc.sync.dma_start(yv[:, :, 0:1], sfull[:, :, NSTEP : NSTEP + 1])
            nc.sync.dma_start(
                yv[:, :, 1:2], vT[:].rearrange("p (b one) -> p b one", one=1)
            )

    _legalize_waits(nc)
    return nc


def _prep_host(inputs):
    rnorm = np.ascontiguousarray(np.asarray(inputs["rnorm"], dtype=np.float32))
    W0 = np.asarray(inputs["W0"], dtype=np.float32)
    b0 = np.asarray(inputs["b0"], dtype=np.float32)
    Wh = np.asarray(inputs["Wh"], dtype=np.float32)
    bh = np.asarray(inputs["bh"], dtype=np.float32)
    Wf = np.asarray(inputs["Wf"], dtype=np.float32)
    bf = np.asarray(inputs["bf"], dtype=np.float32)

    sd_np = mybir.dt.np(SD)

    # tk[p, x*8 + kgp]: t/ones planes; t = DT*(x mod 128) for every p
    tk = np.zeros((P, NX, 8), np.float32)
    n_of_x = np.tile(np.arange(NSTEP, dtype=np.float32), 2)
    for g in range(NG):
        tk[:, :, 0 + g] = (DT * n_of_x)[None, :]
        tk[:, :, 4 + g] = 1.0

    ident = np.eye(P, dtype=np.float32)

    # lhsT rows r = 4k+g; planes k: 0 t, 1 ones, 2 s, 3 Ds, 4 s*r
    # K=128 with 4 quadrant-masked variants per stream: variant xi has the
    # 32-row coef block at partitions 32*xi and zeros elsewhere
    l0 = np.zeros((KG, P), np.float32)
    lg = np.zeros((KG, P), np.float32)
    lu = np.zeros((KG, P), np.float32)
    for g in range(NG):
        cols = slice(32 * g, 32 * (g + 1))
        l0[4 * 0 + g, cols] = W0[:, 0]                          # t
        l0[4 * 2 + g, cols] = W0[:, 1]                          # s
        lg[4 * 1 + g, cols] = W0[:, 0] * DT                     # ones -> dhdt*dt
        lg[4 * 3 + g, cols] = W0[:, 1]                          # Ds
        lu[4 * 4 + g, cols] = W0[:, 1] * SIG * float(np.sqrt(0.5 * DT))
    lhsTL = np.zeros((12, P, P), np.float32)
    for s, blk in enumerate((l0, lg, lu)):
        for xi in range(4):
            lhsTL[s * 4 + xi, 32 * xi : 32 * (xi + 1), :] = blk

    lhsTh = np.zeros((NH, P, P), np.float32)
    for l in range(NH):
        for g in range(NG):
            blk = slice(32 * g, 32 * (g + 1))
            lhsTh[l, blk, blk] = Wh[l].T
    lhsTf = np.zeros((P, NG), np.float32)
    for g in range(NG):
        lhsTf[32 * g : 32 * (g + 1), g] = Wf[0]

    bias = np.zeros((P, 4, 2), np.float32)
    bias[:, 0, 0] = np.tile(b0, NG)
    bias[:, 0, 1] = 0.5 * bias[:, 0, 0]
    for l in range(NH):
        bias[:, l + 1, 0] = np.tile(bh[l], NG)
        bias[:, l + 1, 1] = 0.5 * bias[:, l + 1, 0]
    bfh = np.full((P, 1), 0.5 * bf[0], np.float32)

    shared = {
        "tk": tk.reshape(P, NX * 8).astype(sd_np),
        "ident": ident.astype(sd_np),
        "lhsTL": lhsTL.astype(sd_np),
        "lhsTh": lhsTh.astype(sd_np),
        "lhsTf": lhsTf.astype(sd_np),
        "bias": bias,
        "bfh": bfh,
    }

    in_maps = []
    for core in range(NCORE):
        shard = rnorm[core * BC : (core + 1) * BC]          # [1024, 128]
        sg = np.ascontiguousarray(
            shard.reshape(NB, P, NSTEP).transpose(1, 0, 2).reshape(P, NB * NSTEP)
        )
        in_maps.append({"rn_sg": sg, **shared})
    return in_maps


last_perf = {}


def kernel(trace=False, **inputs) -> np.ndarray:
    if "nc" not in _CACHE:
        _CACHE["nc"] = _build_program()
    nc = _CACHE["nc"]
    in_maps = _prep_host(inputs)
    res = run_bass_kernel_spmd(nc, in_maps, list(range(NCORE)), trace=trace)
    last_perf["exec_time_ns"] = res.exec_time_ns
    out = np.empty((B, 2), np.float32)
    for core in range(NCORE):
        yt = res.results[core]["yT"]                        # [1024, 2]
        out[core * BC : (core + 1) * BC] = yt
    return out


# revision 16
# speedup vs baseline: 3.2266x; 1.0264x over previous
"""Trainium2 Bass kernel for the deep-hedging Milstein SDE loss.

Math: the reference scan collapses (see derivation in comments below):
  s_{n+1} = s_n * m_n,  m_n = c0 + c1*r_n + c2*r_n^2
  v_{n+1} = v_n + sp_n * (Zw_n - Zu_n^2 * Tf_n)            [per-point phi terms]
where the per-point quantities come from a forward-mode jet of the holding
MLP with THREE streams:
  a  : primal silu chain
  u  : first-order tangent along (0, sqrt(0.5*dt)*SIG*s*r)   [2nd-order probe]
  w  : merged gamma + second-order stream:
         w0 = silu'(z)*Mg + silu''(z)*Mu^2
         w' = silu'(z)*Zw + silu''(z)*Zu^2
(the gamma direction is (dt, Ds); gamma and the 2nd-order stream propagate
with the same linear rule and are only ever used summed, so they merge.)

Layout per core (1024 paths, 128 steps):
  sgrid [128 part = p, 8 blocks b, 128 steps n], path_local = b*128 + p.
  MLP groups g = b // 2 (4 groups of 2 blocks); point column within a group:
      j = x*128 + p,   x = b2*128 + n,  b = 2*g + b2.
  Chunk ci = x in [4ci, 4ci+4) -> 512 columns.

Stage B repack is done on the PE: S5T [p, x, kgp(32: 20 real + 12 pad)]
holds the 5 value planes (t, ones, s, Ds, s*r) interleaved so that one
[128,128] PE transpose per chunk yields the matmul rhs [(x4, kg32), p].
A plain DMA cannot do this repack: the cost model charges per-partition
bytes and the BIR verifier requires the partition-crossing dim first on
both sides, which forbids partition-transposing DMAs.

Engine split per chunk (V1 CoreSim cost model):
  PE  : 1 transpose + 12 L0 sub-matmuls + 9 hidden + 3 final  (~3.3us)
  ACT : silu' + tanh per layer (+1 staging copy)              (~5.5-6.1us)
  DVE : all-SBUF f16 stt/ts ops at 4x mode (193ns each)       (~5.7us)
  Pool: PSUM-reading stt ops at flat 427ns                    (~6.1us)
"""

import os

import numpy as np

import concourse.bass as bass
import concourse.mybir as mybir
from concourse import tile
from concourse.bass_utils import run_bass_kernel_spmd


# problem constants (hardcoded per spec)
B = 8192
NSTEP = 128
NCORE = 8
BC = B // NCORE          # 1024 paths per core
P = 128                  # partitions
NB = BC // P             # 8 path blocks
WIDTH = 32
NG = 4                   # feature groups on partitions
NH = 3                   # hidden layers
NX = 2 * NSTEP           # 256 x-values (b2, n)
C = NX * P               # 32768 point-columns per group
CC = 512                 # chunk columns (4 x-values * 128 p)
NCHUNK = NX // 4         # 64
KREAL = 20               # 5 value planes * 4 groups
KG = 32                  # padded plane rows per x in S5T
T0, T1 = 0.0, 1.0
MU, SIG = 1.0, 1.0
DT = (T1 - T0) / NSTEP
SQDT = float(np.sqrt(DT))

F32 = mybir.dt.float32
AF = mybir.ActivationFunctionType
ALU = mybir.AluOpType

SD = mybir.dt.float16

_CACHE = {}
DBG_NCHUNK = int(os.environ.get("KDBG_NCHUNK", "0")) or None


def _legalize_waits(nc):
    """Split long on_wait lists into standalone single-wait NoOps.

    This walrus rejects instructions whose sync_info carries more waits
    than the ISA encoding holds. Tile emits up to one wait per logical
    processor, so spill the excess onto NoOps on the same engine queue,
    which execute in order before the real instruction.
    """
    ctr = 0
    for bb in nc.main_func.blocks:
        out = []
        for ins in bb.instructions:
            si = ins.sync_info
            if si is not None and si.on_wait:
                limit = 1
                waits = list(si.on_wait)
                if len(waits) > limit:
                    spill, keep = waits[:-limit], waits[-limit:]
                    for w in spill:
                        ctr += 1
                        nop = mybir.InstNoOp(name=f"waitnop_{ctr}", ins=[], outs=[])
                        nop.engine = ins.engine
                        nop.sync_info = mybir.SyncInfo(on_wait=[w], on_update=[])
                        out.append(nop)
                    si.on_wait = keep
            out.append(ins)
        bb.instructions = out


def _build_program():
    nc = bass.Bass()

    rn_d = nc.declare_dram_parameter("rn_sg", [P, NB * NSTEP], F32, isOutput=False)
    tk_d = nc.declare_dram_parameter("tk", [P, NX * 8], SD, isOutput=False)
    id_d = nc.declare_dram_parameter("ident", [P, P], SD, isOutput=False)
    lhsTL_d = nc.declare_dram_parameter("lhsTL", [12, P, P], SD, isOutput=False)
    lhsTh_d = nc.declare_dram_parameter("lhsTh", [NH, P, P], SD, isOutput=False)
    lhsTf_d = nc.declare_dram_parameter("lhsTf", [P, NG], SD, isOutput=False)
    bias_d = nc.declare_dram_parameter("bias", [P, 4, 2], F32, isOutput=False)
    bfh_d = nc.declare_dram_parameter("bfh", [P, 1], F32, isOutput=False)
    out_d = nc.declare_dram_parameter("yT", [BC, 2], F32, isOutput=True)

    # m_n = c0 + c1*r + c2*r^2
    c0 = 1.0 + MU * DT - 0.5 * SIG * SIG * DT
    c1 = SIG * SQDT
    c2 = 0.5 * SIG * SIG * DT

    with tile.TileContext(nc) as tc:
        with (
            tc.tile_pool(name="const", bufs=1) as cpool,
            tc.tile_pool(name="sg", bufs=1) as sgpool,
            tc.tile_pool(name="work", bufs=6) as wpool,
            tc.tile_pool(name="stream", bufs=10) as spool,
            tc.tile_pool(name="psum", bufs=7, space="PSUM") as pspool,
            tc.tile_pool(name="pst", bufs=1, space="PSUM") as pstpool,
        ):
            # ---- constants ----
            ident = cpool.tile([P, P], SD, tag="ident")
            lhsTL = [
                cpool.tile([P, P], SD, tag=f"lhsTL{i}", name=f"lhsTL{i}")
                for i in range(12)
            ]
            lhsTh = [
                cpool.tile([P, P], SD, tag=f"lhsTh{l}", name=f"lhsTh{l}")
                for l in range(NH)
            ]
            lhsTf = cpool.tile([P, NG], SD, tag="lhsTf")
            bias = cpool.tile([P, 4, 2], F32, tag="bias")
            bfh = cpool.tile([P, 1], F32, tag="bfh")
            nc.sync.dma_start(ident[:], id_d[:])
            for i in range(12):
                nc.scalar.dma_start(lhsTL[i][:], lhsTL_d[i])
            for l in range(NH):
                nc.scalar.dma_start(lhsTh[l][:], lhsTh_d[l])
            nc.scalar.dma_start(lhsTf[:], lhsTf_d[:])
            nc.sync.dma_start(bias[:], bias_d[:])
            nc.sync.dma_start(bfh[:], bfh_d[:])

            def bias_r(l, h):
                return bias[:, l, h : h + 1]

            # ---- stage A: sgrid GBM math -> S5T staging ----
            # S5T[p, x, kgp]: kgp = 4k+g; planes k: 0 t, 1 ones, 2 s, 3 Ds, 4 s*r
            S5T = sgpool.tile([P, NX, KG], SD, tag="S5T")
            rs = sgpool.tile([P, NB, NSTEP], F32, tag="rs")
            nc.sync.dma_start(rs[:], rn_d[:].rearrange("p (b n) -> p b n", b=NB))
            # constant planes (t, ones) from DRAM
            nc.sync.dma_start(
                S5T[:, :, 0:8],
                tk_d[:].rearrange("p (x k) -> p x k", k=8),
            )
            # pad rows: keep finite for the transpose passthrough
            nc.gpsimd.memset(S5T[:, :, KREAL:KG], 0.0)
            scr = sgpool.tile([P, NB, NSTEP], F32, tag="scr")
            m = sgpool.tile([P, NB, NSTEP], F32, tag="m")
            # m = (c2*r + c1)*r + c0
            nc.vector.tensor_scalar(scr[:], rs[:], c2, c1, ALU.mult, ALU.add)
            nc.vector.scalar_tensor_tensor(m[:], scr[:], 0.0, rs[:], ALU.add, ALU.mult)
            nc.vector.tensor_scalar(m[:], m[:], 1.0, c0, ALU.mult, ALU.add)

            sfull = sgpool.tile([P, NB, NSTEP + 1], F32, tag="sfull")
            nc.vector.memset(sfull[:, :, 0:1], 1.0)
            for b in range(NB):
                nc.vector.tensor_tensor_scan(
                    sfull[:, b, 1 : NSTEP + 1],
                    m[:, b, :],
                    m[:, b, :],
                    1.0,
                    ALU.mult,
                    ALU.bypass,
                )
            sN = sfull[:, :, 0:NSTEP]

            # plane views into S5T: iteration (p, g, b2, n) matching sgrid (p, b=2g+b2, n)
            def plane(k):
                return S5T[:].rearrange("p (b2 n) (k g) -> k p g b2 n", k=8, b2=2)[k]

            def sg_gb(t_ap):
                # sgrid [p, b, n] -> [p, g, b2, n]
                return t_ap.rearrange("p (g b2) n -> p g b2 n", g=NG)

            # s plane (Pool), Ds plane (DVE), s*r plane (Pool)
            nc.gpsimd.tensor_copy(plane(2), sg_gb(sN))
            nc.vector.scalar_tensor_tensor(
                plane(3), sg_gb(m[:]), 1.0, sg_gb(sN), ALU.subtract, ALU.mult
            )
            nc.gpsimd.tensor_tensor(plane(4), sg_gb(sN), sg_gb(rs[:]), ALU.mult)

            # ---- staging for stage D: rows 32s+g, cols j = x*128+p ----
            staging = sgpool.tile([P, C], SD, tag="staging")

            # ---- stage C: chunked MLP jet (wavefront-pipelined emission) ----
            # Stages per chunk c:
            #   j=0: PE transpose + DVE rhsb copy
            #   j=1..4: layer l=j-1: PE matmuls + ACT (s1, T, zu16) + elementwise
            #   j=5: final matmuls + staging copy
            # Emitting stage j of chunk k-j at iteration k keeps every engine
            # queue filled with ~6 different chunks' ready work (in-order
            # engine queues would otherwise stall on the intra-chunk chain).
            nchunk = DBG_NCHUNK or NCHUNK
            cstate = {}

            def st_transpose(c):
                pst = pstpool.tile([P, P], SD, tag="pst")
                nc.tensor.transpose(pst[:], S5T[:, 4 * c : 4 * c + 4, :], ident[:])
                rhsb = spool.tile([P, P], SD, tag="rhsb")
                if c & 1:
                    nc.vector.tensor_copy(rhsb[:], pst[:])
                else:
                    nc.scalar.activation(rhsb[:], pst[:], AF.Identity)
                cstate[c] = {"rhsb": rhsb}

            def st_layer(c, l):
                S = cstate[c]
                if l == 0:
                    Zp = pspool.tile([P, CC], F32, tag="ps")
                    Zw = pspool.tile([P, CC], F32, tag="ps")
                    Zu = pspool.tile([P, CC], F32, tag="ps")
                    rv = S.pop("rhsb")
                    for xi in range(4):
                        sl = slice(xi * P, (xi + 1) * P)
                        nc.tensor.matmul(Zp[:, sl], lhsTL[0 + xi][:], rv[:], start=True, stop=True)
                        nc.tensor.matmul(Zw[:, sl], lhsTL[4 + xi][:], rv[:], start=True, stop=True)
                        nc.tensor.matmul(Zu[:, sl], lhsTL[8 + xi][:], rv[:], start=True, stop=True)
                else:
                    a_p, u_p, w_p = S.pop("a"), S.pop("u"), S.pop("w")
                    Zp = pspool.tile([P, CC], F32, tag="ps")
                    Zu = pspool.tile([P, CC], F32, tag="ps")
                    Zw = pspool.tile([P, CC], F32, tag="ps")
                    nc.tensor.matmul(Zp[:], lhsTh[l - 1][:], a_p[:], start=True, stop=True)
                    nc.tensor.matmul(Zu[:], lhsTh[l - 1][:], u_p[:], start=True, stop=True)
                    nc.tensor.matmul(Zw[:], lhsTh[l - 1][:], w_p[:], start=True, stop=True)

                s1 = wpool.tile([P, CC], SD, tag="s1")
                nc.scalar.activation(
                    s1[:], Zp[:], AF.Derivative_silu, bias=bias_r(l, 0)
                )
                T = wpool.tile([P, CC], SD, tag="T")
                nc.scalar.activation(
                    T[:], Zp[:], AF.Tanh, bias=bias_r(l, 1), scale=0.5
                )
                zu16 = wpool.tile([P, CC], SD, tag="zu16")
                nc.scalar.activation(zu16[:], Zu[:], AF.Identity)

                # sigma = 0.5*T + 0.5                        (DVE ts 4x)
                sg = wpool.tile([P, CC], SD, tag="sg")
                nc.vector.tensor_scalar(sg[:], T[:], 0.5, 0.5, ALU.mult, ALU.add)
                # a' = (Zp + b) * sigma                      (DVE stt, PSUM)
                a = spool.tile([P, CC], SD, tag="a")
                nc.vector.scalar_tensor_tensor(
                    a[:], Zp[:], bias_r(l, 0), sg[:], ALU.add, ALU.mult
                )
                # q = Zw * s1                                (DVE tt, PSUM)
                q = wpool.tile([P, CC], SD, tag="q")
                nc.vector.tensor_tensor(q[:], Zw[:], s1[:], ALU.mult)
                # u' = zu16 * s1                             (Pool)
                u = spool.tile([P, CC], SD, tag="u")
                nc.gpsimd.tensor_tensor(u[:], zu16[:], s1[:], ALU.mult)
                # A = zu16^2                                 (Pool)
                A = wpool.tile([P, CC], SD, tag="A")
                nc.gpsimd.tensor_tensor(A[:], zu16[:], zu16[:], ALU.mult)
                # P1 = s1*T; D = P1 - sigma = -silu''       (DVE tt f16 2x)
                P1 = wpool.tile([P, CC], SD, tag="P1")
                nc.vector.tensor_tensor(P1[:], s1[:], T[:], ALU.mult)
                D = wpool.tile([P, CC], SD, tag="D")
                deng = nc.vector if l == 0 else nc.gpsimd
                deng.tensor_tensor(D[:], P1[:], sg[:], ALU.subtract)
                # phi = D*A = -silu''*A                      (Pool)
                ph = wpool.tile([P, CC], SD, tag="ph")
                nc.gpsimd.tensor_tensor(ph[:], D[:], A[:], ALU.mult)
                # w' = q - phi = q + silu''*A                (Pool)
                w = spool.tile([P, CC], SD, tag="w")
                nc.gpsimd.tensor_tensor(w[:], q[:], ph[:], ALU.subtract)
                S["a"], S["u"], S["w"] = a, u, w

            def st_final(c):
                S = cstate.pop(c)
                a, u, w = S["a"], S["u"], S["w"]
                F = pspool.tile([P, CC], F32, tag="ps")
                nc.tensor.matmul(F[0:4, :], lhsTf[:], a[:], start=True, stop=True)
                nc.tensor.matmul(F[32:36, :], lhsTf[:], u[:], start=True, stop=True)
                nc.tensor.matmul(F[64:68, :], lhsTf[:], w[:], start=True, stop=True)
                dst = staging[0:68, :].rearrange("r (p x) -> r p x", p=P)[
                    :, :, 4 * c : 4 * c + 4
                ]
                fin = F[0:68, :].rearrange("r (xi p) -> r p xi", xi=4)
                if c % 3 == 2:
                    nc.vector.tensor_copy(dst, fin)
                else:
                    nc.scalar.copy(dst, fin)

            for k in range(nchunk + 6):
                for j in range(6):
                    c = k - j
                    if not (0 <= c < nchunk):
                        continue
                    if j == 0:
                        st_transpose(c)
                    elif j < 5:
                        st_layer(c, j - 1)
                    else:
                        st_final(c)

            # ---- stage D: repack + phi assembly + reduction ----
            zf_sg = sgpool.tile([P, NB, NSTEP], SD, tag="zf_sg")
            uf_sg = sgpool.tile([P, NB, NSTEP], SD, tag="uf_sg")
            wf_sg = sgpool.tile([P, NB, NSTEP], SD, tag="wf_sg")
            if DBG_NCHUNK is None:
                for s, sgt in enumerate((zf_sg, uf_sg, wf_sg)):
                    for g in range(NG):
                        row = 32 * s + g
                        src = staging[row : row + 1, :].rearrange(
                            "one (p x) -> one p x", p=P
                        )
                        dst = sgt[:, 2 * g : 2 * g + 2, :].rearrange(
                            "p b2 n -> p (b2 n)"
                        )
                        nc.sync.dma_start(dst, src)

                Tf = sgpool.tile([P, NB, NSTEP], SD, tag="Tf")
                nc.scalar.activation(Tf[:], zf_sg[:], AF.Tanh, bias=bfh[:], scale=0.5)
                U2 = sgpool.tile([P, NB, NSTEP], SD, tag="U2")
                nc.vector.scalar_tensor_tensor(
                    U2[:], uf_sg[:], 0.0, uf_sg[:], ALU.add, ALU.mult
                )
                Q = sgpool.tile([P, NB, NSTEP], SD, tag="Q")
                nc.vector.scalar_tensor_tensor(
                    Q[:], U2[:], 0.0, Tf[:], ALU.add, ALU.mult
                )
                Dd = sgpool.tile([P, NB, NSTEP], SD, tag="Dd")
                nc.vector.scalar_tensor_tensor(
                    Dd[:], wf_sg[:], 0.0, Q[:], ALU.add, ALU.subtract
                )
                T2 = sgpool.tile([P, NB, NSTEP], SD, tag="T2")
                nc.vector.scalar_tensor_tensor(
                    T2[:], Tf[:], 0.0, Tf[:], ALU.add, ALU.mult
                )
                sp = sgpool.tile([P, NB, NSTEP], SD, tag="sp")
                nc.vector.tensor_scalar(sp[:], T2[:], -0.25, 0.25, ALU.mult, ALU.add)
                Sd = sgpool.tile([P, NB, NSTEP], SD, tag="Sd")
                vT = sgpool.tile([P, NB], F32, tag="vT")
                for b in range(NB):
                    nc.vector.scalar_tensor_tensor(
                        Sd[:, b, :],
                        Dd[:, b, :],
                        0.0,
                        sp[:, b, :],
                        ALU.add,
                        ALU.mult,
                        accum_out=vT[:, b : b + 1],
                    )
            else:
                vT = sgpool.tile([P, NB], F32, tag="vT")
                nc.vector.memset(vT[:], 0.0)

            # ---- outputs ----
            yv = out_d[:].rearrange("(b p) c -> p b c", p=P)
            nc.sync.dma_start(yv[:, :, 0:1], sfull[:, :, NSTEP : NSTEP + 1])
            nc.sync.dma_start(
                yv[:, :, 1:2], vT[:].rearrange("p (b one) -> p b one", one=1)
            )

    _legalize_waits(nc)
    return nc


def _prep_host(inputs):
    rnorm = np.ascontiguousarray(np.asarray(inputs["rnorm"], dtype=np.float32))
    W0 = np.asarray(inputs["W0"], dtype=np.float32)
    b0 = np.asarray(inputs["b0"], dtype=np.float32)
    Wh = np.asarray(inputs["Wh"], dtype=np.float32)
    bh = np.asarray(inputs["bh"], dtype=np.float32)
    Wf = np.asarray(inputs["Wf"], dtype=np.float32)
    bf = np.asarray(inputs["bf"], dtype=np.float32)

    sd_np = mybir.dt.np(SD)

    # tk[p, x*8 + kgp]: t/ones planes; t = DT*(x mod 128) for every p
    tk = np.zeros((P, NX, 8), np.float32)
    n_of_x = np.tile(np.arange(NSTEP, dtype=np.float32), 2)
    for g in range(NG):
        tk[:, :, 0 + g] = (DT * n_of_x)[None, :]
        tk[:, :, 4 + g] = 1.0

    ident = np.eye(P, dtype=np.float32)

    # lhsT rows r = 4k+g; planes k: 0 t, 1 ones, 2 s, 3 Ds, 4 s*r
    # K=128 with 4 quadrant-masked variants per stream: variant xi has the
    # 32-row coef block at partitions 32*xi and zeros elsewhere
    l0 = np.zeros((KG, P), np.float32)
    lg = np.zeros((KG, P), np.float32)
    lu = np.zeros((KG, P), np.float32)
    for g in range(NG):
        cols = slice(32 * g, 32 * (g + 1))
        l0[4 * 0 + g, cols] = W0[:, 0]                          # t
        l0[4 * 2 + g, cols] = W0[:, 1]                          # s
        lg[4 * 1 + g, cols] = W0[:, 0] * DT                     # ones -> dhdt*dt
        lg[4 * 3 + g, cols] = W0[:, 1]                          # Ds
        lu[4 * 4 + g, cols] = W0[:, 1] * SIG * float(np.sqrt(0.5 * DT))
    lhsTL = np.zeros((12, P, P), np.float32)
    for s, blk in enumerate((l0, lg, lu)):
        for xi in range(4):
            lhsTL[s * 4 + xi, 32 * xi : 32 * (xi + 1), :] = blk

    lhsTh = np.zeros((NH, P, P), np.float32)
    for l in range(NH):
        for g in range(NG):
            blk = slice(32 * g, 32 * (g + 1))
            lhsTh[l, blk, blk] = Wh[l].T
    lhsTf = np.zeros((P, NG), np.float32)
    for g in range(NG):
        lhsTf[32 * g : 32 * (g + 1), g] = Wf[0]

    bias = np.zeros((P, 4, 2), np.float32)
    bias[:, 0, 0] = np.tile(b0, NG)
    bias[:, 0, 1] = 0.5 * bias[:, 0, 0]
    for l in range(NH):
        bias[:, l + 1, 0] = np.tile(bh[l], NG)
        bias[:, l + 1, 1] = 0.5 * bias[:, l + 1, 0]
    bfh = np.full((P, 1), 0.5 * bf[0], np.float32)

    shared = {
        "tk": tk.reshape(P, NX * 8).astype(sd_np),
        "ident": ident.astype(sd_np),
        "lhsTL": lhsTL.astype(sd_np),
        "lhsTh": lhsTh.astype(sd_np),
        "lhsTf": lhsTf.astype(sd_np),
        "bias": bias,
        "bfh": bfh,
    }

    in_maps = []
    for core in range(NCORE):
        shard = rnorm[core * BC : (core + 1) * BC]          # [1024, 128]
        sg = np.ascontiguousarray(
            shard.reshape(NB, P, NSTEP).transpose(1, 0, 2).reshape(P, NB * NSTEP)
        )
        in_maps.append({"rn_sg": sg, **shared})
    return in_maps


last_perf = {}


def kernel(trace=False, **inputs) -> np.ndarray:
    if "nc" not in _CACHE:
        _CACHE["nc"] = _build_program()
    nc = _CACHE["nc"]
    in_maps = _prep_host(inputs)
    res = run_bass_kernel_spmd(nc, in_maps, list(range(NCORE)), trace=trace)
    last_perf["exec_time_ns"] = res.exec_time_ns
    out = np.empty((B, 2), np.float32)
    for core in range(NCORE):
        yt = res.results[core]["yT"]                        # [1024, 2]
        out[core * BC : (core + 1) * BC] = yt
    return out


# revision 17
# speedup vs baseline: 3.3227x; 1.0298x over previous
"""Trainium2 Bass kernel for the deep-hedging Milstein SDE loss.

Math: the reference scan collapses (see derivation in comments below):
  s_{n+1} = s_n * m_n,  m_n = c0 + c1*r_n + c2*r_n^2
  v_{n+1} = v_n + sp_n * (Zw_n - Zu_n^2 * Tf_n)            [per-point phi terms]
where the per-point quantities come from a forward-mode jet of the holding
MLP with THREE streams:
  a  : primal silu chain
  u  : first-order tangent along (0, sqrt(0.5*dt)*SIG*s*r)   [2nd-order probe]
  w  : merged gamma + second-order stream:
         w0 = silu'(z)*Mg + silu''(z)*Mu^2
         w' = silu'(z)*Zw + silu''(z)*Zu^2
(the gamma direction is (dt, Ds); gamma and the 2nd-order stream propagate
with the same linear rule and are only ever used summed, so they merge.)

Layout per core (1024 paths, 128 steps):
  sgrid [128 part = p, 8 blocks b, 128 steps n], path_local = b*128 + p.
  MLP groups g = b // 2 (4 groups of 2 blocks); point column within a group:
      j = x*128 + p,   x = b2*128 + n,  b = 2*g + b2.
  Chunk ci = x in [4ci, 4ci+4) -> 512 columns.

Stage B repack is done on the PE: S5T [p, x, kgp(32: 20 real + 12 pad)]
holds the 5 value planes (t, ones, s, Ds, s*r) interleaved so that one
[128,128] PE transpose per chunk yields the matmul rhs [(x4, kg32), p].
A plain DMA cannot do this repack: the cost model charges per-partition
bytes and the BIR verifier requires the partition-crossing dim first on
both sides, which forbids partition-transposing DMAs.

Engine split per chunk (V1 CoreSim cost model):
  PE  : 1 transpose + 12 L0 sub-matmuls + 9 hidden + 3 final  (~3.3us)
  ACT : silu' + tanh per layer (+1 staging copy)              (~5.5-6.1us)
  DVE : all-SBUF f16 stt/ts ops at 4x mode (193ns each)       (~5.7us)
  Pool: PSUM-reading stt ops at flat 427ns                    (~6.1us)
"""

import os

import numpy as np

import concourse.bass as bass
import concourse.mybir as mybir
from concourse import tile
from concourse.bass_utils import run_bass_kernel_spmd


# problem constants (hardcoded per spec)
B = 8192
NSTEP = 128
NCORE = 8
BC = B // NCORE          # 1024 paths per core
P = 128                  # partitions
NB = BC // P             # 8 path blocks
WIDTH = 32
NG = 4                   # feature groups on partitions
NH = 3                   # hidden layers
NX = 2 * NSTEP           # 256 x-values (b2, n)
C = NX * P               # 32768 point-columns per group
CC = 512                 # chunk columns (4 x-values * 128 p)
NCHUNK = NX // 4         # 64
KREAL = 20               # 5 value planes * 4 groups
KG = 32                  # padded plane rows per x in S5T
T0, T1 = 0.0, 1.0
MU, SIG = 1.0, 1.0
DT = (T1 - T0) / NSTEP
SQDT = float(np.sqrt(DT))

F32 = mybir.dt.float32
AF = mybir.ActivationFunctionType
ALU = mybir.AluOpType

SD = mybir.dt.float16

_CACHE = {}
DBG_NCHUNK = int(os.environ.get("KDBG_NCHUNK", "0")) or None


def _legalize_waits(nc):
    """Split long on_wait lists into standalone single-wait NoOps.

    This walrus rejects instructions whose sync_info carries more waits
    than the ISA encoding holds. Tile emits up to one wait per logical
    processor, so spill the excess onto NoOps on the same engine queue,
    which execute in order before the real instruction.
    """
    ctr = 0
    for bb in nc.main_func.blocks:
        out = []
        for ins in bb.instructions:
            si = ins.sync_info
            if si is not None and si.on_wait:
                limit = 1
                waits = list(si.on_wait)
                if len(waits) > limit:
                    spill, keep = waits[:-limit], waits[-limit:]
                    for w in spill:
                        ctr += 1
                        nop = mybir.InstNoOp(name=f"waitnop_{ctr}", ins=[], outs=[])
                        nop.engine = ins.engine
                        nop.sync_info = mybir.SyncInfo(on_wait=[w], on_update=[])
                        out.append(nop)
                    si.on_wait = keep
            out.append(ins)
        bb.instructions = out


def _build_program():
    nc = bass.Bass()

    rn_d = nc.declare_dram_parameter("rn_sg", [P, NB * NSTEP], F32, isOutput=False)
    tk_d = nc.declare_dram_parameter("tk", [P, NX * 8], SD, isOutput=False)
    id_d = nc.declare_dram_parameter("ident", [P, P], SD, isOutput=False)
    lhsTL_d = nc.declare_dram_parameter("lhsTL", [12, P, P], SD, isOutput=False)
    lhsTh_d = nc.declare_dram_parameter("lhsTh", [NH, P, P], SD, isOutput=False)
    lhsThN_d = nc.declare_dram_parameter("lhsThN", [NH, P, P], SD, isOutput=False)
    lhsTf_d = nc.declare_dram_parameter("lhsTf", [P, NG], SD, isOutput=False)
    lhsTfN_d = nc.declare_dram_parameter("lhsTfN", [P, NG], SD, isOutput=False)
    bias_d = nc.declare_dram_parameter("bias", [P, 4, 2], F32, isOutput=False)
    bfh_d = nc.declare_dram_parameter("bfh", [P, 1], F32, isOutput=False)
    out_d = nc.declare_dram_parameter("yT", [BC, 2], F32, isOutput=True)

    # m_n = c0 + c1*r + c2*r^2
    c0 = 1.0 + MU * DT - 0.5 * SIG * SIG * DT
    c1 = SIG * SQDT
    c2 = 0.5 * SIG * SIG * DT

    with tile.TileContext(nc) as tc:
        with (
            tc.tile_pool(name="const", bufs=1) as cpool,
            tc.tile_pool(name="sg", bufs=1) as sgpool,
            tc.tile_pool(name="work", bufs=6) as wpool,
            tc.tile_pool(name="stream", bufs=10) as spool,
            tc.tile_pool(name="psum", bufs=7, space="PSUM") as pspool,
            tc.tile_pool(name="pst", bufs=1, space="PSUM") as pstpool,
        ):
            # ---- constants ----
            ident = cpool.tile([P, P], SD, tag="ident")
            lhsTL = [
                cpool.tile([P, P], SD, tag=f"lhsTL{i}", name=f"lhsTL{i}")
                for i in range(12)
            ]
            lhsTh = [
                cpool.tile([P, P], SD, tag=f"lhsTh{l}", name=f"lhsTh{l}")
                for l in range(NH)
            ]
            lhsThN = [
                cpool.tile([P, P], SD, tag=f"lhsThN{l}", name=f"lhsThN{l}")
                for l in range(NH)
            ]
            lhsTf = cpool.tile([P, NG], SD, tag="lhsTf")
            lhsTfN = cpool.tile([P, NG], SD, tag="lhsTfN")
            bias = cpool.tile([P, 4, 2], F32, tag="bias")
            bfh = cpool.tile([P, 1], F32, tag="bfh")
            nc.sync.dma_start(ident[:], id_d[:])
            for i in range(12):
                nc.scalar.dma_start(lhsTL[i][:], lhsTL_d[i])
            for l in range(NH):
                nc.scalar.dma_start(lhsTh[l][:], lhsTh_d[l])
                nc.scalar.dma_start(lhsThN[l][:], lhsThN_d[l])
            nc.scalar.dma_start(lhsTf[:], lhsTf_d[:])
            nc.scalar.dma_start(lhsTfN[:], lhsTfN_d[:])
            nc.sync.dma_start(bias[:], bias_d[:])
            nc.sync.dma_start(bfh[:], bfh_d[:])

            def bias_r(l, h):
                return bias[:, l, h : h + 1]

            # ---- stage A: sgrid GBM math -> S5T staging ----
            # S5T[p, x, kgp]: kgp = 4k+g; planes k: 0 t, 1 ones, 2 s, 3 Ds, 4 s*r
            S5T = sgpool.tile([P, NX, KG], SD, tag="S5T")
            rs = sgpool.tile([P, NB, NSTEP], F32, tag="rs")
            nc.sync.dma_start(rs[:], rn_d[:].rearrange("p (b n) -> p b n", b=NB))
            # constant planes (t, ones) from DRAM
            nc.sync.dma_start(
                S5T[:, :, 0:8],
                tk_d[:].rearrange("p (x k) -> p x k", k=8),
            )
            # pad rows: keep finite for the transpose passthrough
            nc.gpsimd.memset(S5T[:, :, KREAL:KG], 0.0)
            scr = sgpool.tile([P, NB, NSTEP], F32, tag="scr")
            m = sgpool.tile([P, NB, NSTEP], F32, tag="m")
            # m = (c2*r + c1)*r + c0
            nc.vector.tensor_scalar(scr[:], rs[:], c2, c1, ALU.mult, ALU.add)
            nc.vector.scalar_tensor_tensor(m[:], scr[:], 0.0, rs[:], ALU.add, ALU.mult)
            nc.vector.tensor_scalar(m[:], m[:], 1.0, c0, ALU.mult, ALU.add)

            sfull = sgpool.tile([P, NB, NSTEP + 1], F32, tag="sfull")
            nc.vector.memset(sfull[:, :, 0:1], 1.0)
            for b in range(NB):
                nc.vector.tensor_tensor_scan(
                    sfull[:, b, 1 : NSTEP + 1],
                    m[:, b, :],
                    m[:, b, :],
                    1.0,
                    ALU.mult,
                    ALU.bypass,
                )
            sN = sfull[:, :, 0:NSTEP]

            # plane views into S5T: iteration (p, g, b2, n) matching sgrid (p, b=2g+b2, n)
            def plane(k):
                return S5T[:].rearrange("p (b2 n) (k g) -> k p g b2 n", k=8, b2=2)[k]

            def sg_gb(t_ap):
                # sgrid [p, b, n] -> [p, g, b2, n]
                return t_ap.rearrange("p (g b2) n -> p g b2 n", g=NG)

            # s plane (Pool), Ds plane (DVE), s*r plane (Pool)
            nc.gpsimd.tensor_copy(plane(2), sg_gb(sN))
            nc.vector.scalar_tensor_tensor(
                plane(3), sg_gb(m[:]), 1.0, sg_gb(sN), ALU.subtract, ALU.mult
            )
            nc.gpsimd.tensor_tensor(plane(4), sg_gb(sN), sg_gb(rs[:]), ALU.mult)

            # ---- staging for stage D: rows 32s+g, cols j = x*128+p ----
            staging = sgpool.tile([P, C], SD, tag="staging")

            # ---- stage C: chunked MLP jet (wavefront-pipelined emission) ----
            # Stages per chunk c:
            #   j=0: PE transpose + DVE rhsb copy
            #   j=1..4: layer l=j-1: PE matmuls + ACT (s1, T, zu16) + elementwise
            #   j=5: final matmuls + staging copy
            # Emitting stage j of chunk k-j at iteration k keeps every engine
            # queue filled with ~6 different chunks' ready work (in-order
            # engine queues would otherwise stall on the intra-chunk chain).
            nchunk = DBG_NCHUNK or NCHUNK
            cstate = {}

            def st_transpose(c):
                pst = pstpool.tile([P, P], SD, tag="pst")
                nc.tensor.transpose(pst[:], S5T[:, 4 * c : 4 * c + 4, :], ident[:])
                rhsb = spool.tile([P, P], SD, tag="rhsb")
                if c & 1:
                    nc.vector.tensor_copy(rhsb[:], pst[:])
                else:
                    nc.scalar.activation(rhsb[:], pst[:], AF.Identity)
                cstate[c] = {"rhsb": rhsb}

            def st_layer(c, l):
                S = cstate[c]
                if l == 0:
                    Zp = pspool.tile([P, CC], F32, tag="ps")
                    Zw = pspool.tile([P, CC], F32, tag="ps")
                    Zu = pspool.tile([P, CC], F32, tag="ps")
                    rv = S.pop("rhsb")
                    for xi in range(4):
                        sl = slice(xi * P, (xi + 1) * P)
                        nc.tensor.matmul(Zp[:, sl], lhsTL[0 + xi][:], rv[:], start=True, stop=True)
                        nc.tensor.matmul(Zw[:, sl], lhsTL[4 + xi][:], rv[:], start=True, stop=True)
                        nc.tensor.matmul(Zu[:, sl], lhsTL[8 + xi][:], rv[:], start=True, stop=True)
                else:
                    a_p, u_p = S.pop("a"), S.pop("u")
                    q_p, ph_p = S.pop("q"), S.pop("ph")
                    Zp = pspool.tile([P, CC], F32, tag="ps")
                    Zu = pspool.tile([P, CC], F32, tag="ps")
                    Zw = pspool.tile([P, CC], F32, tag="ps")
                    nc.tensor.matmul(Zp[:], lhsTh[l - 1][:], a_p[:], start=True, stop=True)
                    nc.tensor.matmul(Zu[:], lhsTh[l - 1][:], u_p[:], start=True, stop=True)
                    # w = q - ph folded into the matmul: Zw = W*q + (-W)*ph
                    nc.tensor.matmul(Zw[:], lhsTh[l - 1][:], q_p[:], start=True, stop=False)
                    nc.tensor.matmul(Zw[:], lhsThN[l - 1][:], ph_p[:], start=False, stop=True)

                s1 = wpool.tile([P, CC], SD, tag="s1")
                nc.scalar.activation(
                    s1[:], Zp[:], AF.Derivative_silu, bias=bias_r(l, 0)
                )
                T = wpool.tile([P, CC], SD, tag="T")
                nc.scalar.activation(
                    T[:], Zp[:], AF.Tanh, bias=bias_r(l, 1), scale=0.5
                )
                zu16 = wpool.tile([P, CC], SD, tag="zu16")
                nc.scalar.activation(zu16[:], Zu[:], AF.Identity)

                # sigma = 0.5*T + 0.5                        (DVE ts 4x)
                sg = wpool.tile([P, CC], SD, tag="sg")
                nc.vector.tensor_scalar(sg[:], T[:], 0.5, 0.5, ALU.mult, ALU.add)
                # a' = (Zp + b) * sigma                      (DVE stt, PSUM)
                a = spool.tile([P, CC], SD, tag="a")
                nc.vector.scalar_tensor_tensor(
                    a[:], Zp[:], bias_r(l, 0), sg[:], ALU.add, ALU.mult
                )
                # q = Zw * s1                                (DVE tt, PSUM)
                q = spool.tile([P, CC], SD, tag="q")
                nc.vector.tensor_tensor(q[:], Zw[:], s1[:], ALU.mult)
                # u' = zu16 * s1                             (Pool)
                u = spool.tile([P, CC], SD, tag="u")
                nc.gpsimd.tensor_tensor(u[:], zu16[:], s1[:], ALU.mult)
                # A = zu16^2                                 (Pool)
                A = wpool.tile([P, CC], SD, tag="A")
                nc.gpsimd.tensor_tensor(A[:], zu16[:], zu16[:], ALU.mult)
                # P1 = s1*T; D = P1 - sigma = -silu''
                P1 = wpool.tile([P, CC], SD, tag="P1")
                peng = nc.vector if l < 2 else nc.gpsimd
                peng.tensor_tensor(P1[:], s1[:], T[:], ALU.mult)
                D = wpool.tile([P, CC], SD, tag="D")
                nc.gpsimd.tensor_tensor(D[:], P1[:], sg[:], ALU.subtract)
                # phi = D*A = -silu''*A  (w' = q - phi folds into next matmul)
                ph = spool.tile([P, CC], SD, tag="ph")
                nc.gpsimd.tensor_tensor(ph[:], D[:], A[:], ALU.mult)
                S["a"], S["u"], S["q"], S["ph"] = a, u, q, ph

            def st_final(c):
                S = cstate.pop(c)
                a, u, q, ph = S["a"], S["u"], S["q"], S["ph"]
                F = pspool.tile([P, CC], F32, tag="ps")
                nc.tensor.matmul(F[0:4, :], lhsTf[:], a[:], start=True, stop=True)
                nc.tensor.matmul(F[32:36, :], lhsTf[:], u[:], start=True, stop=True)
                nc.tensor.matmul(F[64:68, :], lhsTf[:], q[:], start=True, stop=False)
                nc.tensor.matmul(F[64:68, :], lhsTfN[:], ph[:], start=False, stop=True)
                dst = staging[0:68, :].rearrange("r (p x) -> r p x", p=P)[
                    :, :, 4 * c : 4 * c + 4
                ]
                fin = F[0:68, :].rearrange("r (xi p) -> r p xi", xi=4)
                if c % 3 == 2:
                    nc.vector.tensor_copy(dst, fin)
                else:
                    nc.scalar.copy(dst, fin)

            for k in range(nchunk + 6):
                for j in range(6):
                    c = k - j
                    if not (0 <= c < nchunk):
                        continue
                    if j == 0:
                        st_transpose(c)
                    elif j < 5:
                        st_layer(c, j - 1)
                    else:
                        st_final(c)

            # ---- stage D: repack + phi assembly + reduction ----
            zf_sg = sgpool.tile([P, NB, NSTEP], SD, tag="zf_sg")
            uf_sg = sgpool.tile([P, NB, NSTEP], SD, tag="uf_sg")
            wf_sg = sgpool.tile([P, NB, NSTEP], SD, tag="wf_sg")
            if DBG_NCHUNK is None:
                for s, sgt in enumerate((zf_sg, uf_sg, wf_sg)):
                    for g in range(NG):
                        row = 32 * s + g
                        src = staging[row : row + 1, :].rearrange(
                            "one (p x) -> one p x", p=P
                        )
                        dst = sgt[:, 2 * g : 2 * g + 2, :].rearrange(
                            "p b2 n -> p (b2 n)"
                        )
                        nc.sync.dma_start(dst, src)

                Tf = sgpool.tile([P, NB, NSTEP], SD, tag="Tf")
                nc.scalar.activation(Tf[:], zf_sg[:], AF.Tanh, bias=bfh[:], scale=0.5)
                U2 = sgpool.tile([P, NB, NSTEP], SD, tag="U2")
                nc.vector.scalar_tensor_tensor(
                    U2[:], uf_sg[:], 0.0, uf_sg[:], ALU.add, ALU.mult
                )
                Q = sgpool.tile([P, NB, NSTEP], SD, tag="Q")
                nc.vector.scalar_tensor_tensor(
                    Q[:], U2[:], 0.0, Tf[:], ALU.add, ALU.mult
                )
                Dd = sgpool.tile([P, NB, NSTEP], SD, tag="Dd")
                nc.vector.scalar_tensor_tensor(
                    Dd[:], wf_sg[:], 0.0, Q[:], ALU.add, ALU.subtract
                )
                T2 = sgpool.tile([P, NB, NSTEP], SD, tag="T2")
                nc.vector.scalar_tensor_tensor(
                    T2[:], Tf[:], 0.0, Tf[:], ALU.add, ALU.mult
                )
                sp = sgpool.tile([P, NB, NSTEP], SD, tag="sp")
                nc.vector.tensor_scalar(sp[:], T2[:], -0.25, 0.25, ALU.mult, ALU.add)
                Sd = sgpool.tile([P, NB, NSTEP], SD, tag="Sd")
                vT = sgpool.tile([P, NB], F32, tag="vT")
                for b in range(NB):
                    nc.vector.scalar_tensor_tensor(
                        Sd[:, b, :],
                        Dd[:, b, :],
                        0.0,
                        sp[:, b, :],
                        ALU.add,
                        ALU.mult,
                        accum_out=vT[:, b : b + 1],
                    )
            else:
                vT = sgpool.tile([P, NB], F32, tag="vT")
                nc.vector.memset(vT[:], 0.0)

            # ---- outputs ----
            yv = out_d[:].rearrange("(b p) c -> p b c", p=P)
            nc.sync.dma_start(yv[:, :, 0:1], sfull[:, :, NSTEP : NSTEP + 1])
            nc.sync.dma_start(
                yv[:, :, 1:2], vT[:].rearrange("p (b one) -> p b one", one=1)
            )

    _legalize_waits(nc)
    return nc


def _prep_host(inputs):
    rnorm = np.ascontiguousarray(np.asarray(inputs["rnorm"], dtype=np.float32))
    W0 = np.asarray(inputs["W0"], dtype=np.float32)
    b0 = np.asarray(inputs["b0"], dtype=np.float32)
    Wh = np.asarray(inputs["Wh"], dtype=np.float32)
    bh = np.asarray(inputs["bh"], dtype=np.float32)
    Wf = np.asarray(inputs["Wf"], dtype=np.float32)
    bf = np.asarray(inputs["bf"], dtype=np.float32)

    sd_np = mybir.dt.np(SD)

    # tk[p, x*8 + kgp]: t/ones planes; t = DT*(x mod 128) for every p
    tk = np.zeros((P, NX, 8), np.float32)
    n_of_x = np.tile(np.arange(NSTEP, dtype=np.float32), 2)
    for g in range(NG):
        tk[:, :, 0 + g] = (DT * n_of_x)[None, :]
        tk[:, :, 4 + g] = 1.0

    ident = np.eye(P, dtype=np.float32)

    # lhsT rows r = 4k+g; planes k: 0 t, 1 ones, 2 s, 3 Ds, 4 s*r
    # K=128 with 4 quadrant-masked variants per stream: variant xi has the
    # 32-row coef block at partitions 32*xi and zeros elsewhere
    l0 = np.zeros((KG, P), np.float32)
    lg = np.zeros((KG, P), np.float32)
    lu = np.zeros((KG, P), np.float32)
    for g in range(NG):
        cols = slice(32 * g, 32 * (g + 1))
        l0[4 * 0 + g, cols] = W0[:, 0]                          # t
        l0[4 * 2 + g, cols] = W0[:, 1]                          # s
        lg[4 * 1 + g, cols] = W0[:, 0] * DT                     # ones -> dhdt*dt
        lg[4 * 3 + g, cols] = W0[:, 1]                          # Ds
        lu[4 * 4 + g, cols] = W0[:, 1] * SIG * float(np.sqrt(0.5 * DT))
    lhsTL = np.zeros((12, P, P), np.float32)
    for s, blk in enumerate((l0, lg, lu)):
        for xi in range(4):
            lhsTL[s * 4 + xi, 32 * xi : 32 * (xi + 1), :] = blk

    lhsTh = np.zeros((NH, P, P), np.float32)
    for l in range(NH):
        for g in range(NG):
            blk = slice(32 * g, 32 * (g + 1))
            lhsTh[l, blk, blk] = Wh[l].T
    lhsTf = np.zeros((P, NG), np.float32)
    for g in range(NG):
        lhsTf[32 * g : 32 * (g + 1), g] = Wf[0]
    lhsThN = -lhsTh
    lhsTfN = -lhsTf

    bias = np.zeros((P, 4, 2), np.float32)
    bias[:, 0, 0] = np.tile(b0, NG)
    bias[:, 0, 1] = 0.5 * bias[:, 0, 0]
    for l in range(NH):
        bias[:, l + 1, 0] = np.tile(bh[l], NG)
        bias[:, l + 1, 1] = 0.5 * bias[:, l + 1, 0]
    bfh = np.full((P, 1), 0.5 * bf[0], np.float32)

    shared = {
        "tk": tk.reshape(P, NX * 8).astype(sd_np),
        "ident": ident.astype(sd_np),
        "lhsTL": lhsTL.astype(sd_np),
        "lhsTh": lhsTh.astype(sd_np),
        "lhsThN": lhsThN.astype(sd_np),
        "lhsTf": lhsTf.astype(sd_np),
        "lhsTfN": lhsTfN.astype(sd_np),
        "bias": bias,
        "bfh": bfh,
    }

    in_maps = []
    for core in range(NCORE):
        shard = rnorm[core * BC : (core + 1) * BC]          # [1024, 128]
        sg = np.ascontiguousarray(
            shard.reshape(NB, P, NSTEP).transpose(1, 0, 2).reshape(P, NB * NSTEP)
        )
        in_maps.append({"rn_sg": sg, **shared})
    return in_maps


last_perf = {}


def kernel(trace=False, **inputs) -> np.ndarray:
    if "nc" not in _CACHE:
        _CACHE["nc"] = _build_program()
    nc = _CACHE["nc"]
    in_maps = _prep_host(inputs)
    res = run_bass_kernel_spmd(nc, in_maps, list(range(NCORE)), trace=trace)
    last_perf["exec_time_ns"] = res.exec_time_ns
    out = np.empty((B, 2), np.float32)
    for core in range(NCORE):
        yt = res.results[core]["yT"]                        # [1024, 2]
        out[core * BC : (core + 1) * BC] = yt
    return out


# revision 18
# speedup vs baseline: 3.4023x; 1.0239x over previous
"""Trainium2 Bass kernel for the deep-hedging Milstein SDE loss.

Math: the reference scan collapses (see derivation in comments below):
  s_{n+1} = s_n * m_n,  m_n = c0 + c1*r_n + c2*r_n^2
  v_{n+1} = v_n + sp_n * (Zw_n - Zu_n^2 * Tf_n)            [per-point phi terms]
where the per-point quantities come from a forward-mode jet of the holding
MLP with THREE streams:
  a  : primal silu chain
  u  : first-order tangent along (0, sqrt(0.5*dt)*SIG*s*r)   [2nd-order probe]
  w  : merged gamma + second-order stream:
         w0 = silu'(z)*Mg + silu''(z)*Mu^2
         w' = silu'(z)*Zw + silu''(z)*Zu^2
(the gamma direction is (dt, Ds); gamma and the 2nd-order stream propagate
with the same linear rule and are only ever used summed, so they merge.)

Layout per core (1024 paths, 128 steps):
  sgrid [128 part = p, 8 blocks b, 128 steps n], path_local = b*128 + p.
  MLP groups g = b // 2 (4 groups of 2 blocks); point column within a group:
      j = x*128 + p,   x = b2*128 + n,  b = 2*g + b2.
  Chunk ci = x in [4ci, 4ci+4) -> 512 columns.

Stage B repack is done on the PE: S5T [p, x, kgp(32: 20 real + 12 pad)]
holds the 5 value planes (t, ones, s, Ds, s*r) interleaved so that one
[128,128] PE transpose per chunk yields the matmul rhs [(x4, kg32), p].
A plain DMA cannot do this repack: the cost model charges per-partition
bytes and the BIR verifier requires the partition-crossing dim first on
both sides, which forbids partition-transposing DMAs.

Engine split per chunk (V1 CoreSim cost model):
  PE  : 1 transpose + 12 L0 sub-matmuls + 9 hidden + 3 final  (~3.3us)
  ACT : silu' + tanh per layer (+1 staging copy)              (~5.5-6.1us)
  DVE : all-SBUF f16 stt/ts ops at 4x mode (193ns each)       (~5.7us)
  Pool: PSUM-reading stt ops at flat 427ns                    (~6.1us)
"""

import os

import numpy as np

import concourse.bass as bass
import concourse.mybir as mybir
from concourse import tile
from concourse.bass_utils import run_bass_kernel_spmd


# problem constants (hardcoded per spec)
B = 8192
NSTEP = 128
NCORE = 8
BC = B // NCORE          # 1024 paths per core
P = 128                  # partitions
NB = BC // P             # 8 path blocks
WIDTH = 32
NG = 4                   # feature groups on partitions
NH = 3                   # hidden layers
NX = 2 * NSTEP           # 256 x-values (b2, n)
C = NX * P               # 32768 point-columns per group
CC = 512                 # chunk columns (4 x-values * 128 p)
NCHUNK = NX // 4         # 64
KREAL = 20               # 5 value planes * 4 groups
KG = 32                  # padded plane rows per x in S5T
T0, T1 = 0.0, 1.0
MU, SIG = 1.0, 1.0
DT = (T1 - T0) / NSTEP
SQDT = float(np.sqrt(DT))

F32 = mybir.dt.float32
AF = mybir.ActivationFunctionType
ALU = mybir.AluOpType

SD = mybir.dt.float16

_CACHE = {}
DBG_NCHUNK = int(os.environ.get("KDBG_NCHUNK", "0")) or None


def _legalize_waits(nc):
    """Split long on_wait lists into standalone single-wait NoOps.

    This walrus rejects instructions whose sync_info carries more waits
    than the ISA encoding holds. Tile emits up to one wait per logical
    processor, so spill the excess onto NoOps on the same engine queue,
    which execute in order before the real instruction.
    """
    ctr = 0
    for bb in nc.main_func.blocks:
        out = []
        for ins in bb.instructions:
            si = ins.sync_info
            if si is not None and si.on_wait:
                limit = 1
                waits = list(si.on_wait)
                if len(waits) > limit:
                    spill, keep = waits[:-limit], waits[-limit:]
                    for w in spill:
                        ctr += 1
                        nop = mybir.InstNoOp(name=f"waitnop_{ctr}", ins=[], outs=[])
                        nop.engine = ins.engine
                        nop.sync_info = mybir.SyncInfo(on_wait=[w], on_update=[])
                        out.append(nop)
                    si.on_wait = keep
            out.append(ins)
        bb.instructions = out


def _build_program():
    nc = bass.Bass()

    rn_d = nc.declare_dram_parameter("rn_sg", [P, NB * NSTEP], F32, isOutput=False)
    tk_d = nc.declare_dram_parameter("tk", [P, NX * 8], SD, isOutput=False)
    id_d = nc.declare_dram_parameter("ident", [P, P], SD, isOutput=False)
    lhsTL_d = nc.declare_dram_parameter("lhsTL", [12, P, P], SD, isOutput=False)
    lhsTh_d = nc.declare_dram_parameter("lhsTh", [NH, P, P], SD, isOutput=False)
    lhsThN_d = nc.declare_dram_parameter("lhsThN", [NH, P, P], SD, isOutput=False)
    lhsTf_d = nc.declare_dram_parameter("lhsTf", [P, NG], SD, isOutput=False)
    lhsTfN_d = nc.declare_dram_parameter("lhsTfN", [P, NG], SD, isOutput=False)
    bias_d = nc.declare_dram_parameter("bias", [P, 4, 2], F32, isOutput=False)
    bfh_d = nc.declare_dram_parameter("bfh", [P, 1], F32, isOutput=False)
    out_d = nc.declare_dram_parameter("yT", [BC, 2], F32, isOutput=True)

    # m_n = c0 + c1*r + c2*r^2
    c0 = 1.0 + MU * DT - 0.5 * SIG * SIG * DT
    c1 = SIG * SQDT
    c2 = 0.5 * SIG * SIG * DT

    with tile.TileContext(nc) as tc:
        with (
            tc.tile_pool(name="const", bufs=1) as cpool,
            tc.tile_pool(name="sg", bufs=1) as sgpool,
            tc.tile_pool(name="work", bufs=6) as wpool,
            tc.tile_pool(name="stream", bufs=10) as spool,
            tc.tile_pool(name="psum", bufs=7, space="PSUM") as pspool,
            tc.tile_pool(name="pst", bufs=1, space="PSUM") as pstpool,
        ):
            # ---- constants ----
            ident = cpool.tile([P, P], SD, tag="ident")
            lhsTL = [
                cpool.tile([P, P], SD, tag=f"lhsTL{i}", name=f"lhsTL{i}")
                for i in range(12)
            ]
            lhsTh = [
                cpool.tile([P, P], SD, tag=f"lhsTh{l}", name=f"lhsTh{l}")
                for l in range(NH)
            ]
            lhsThN = [
                cpool.tile([P, P], SD, tag=f"lhsThN{l}", name=f"lhsThN{l}")
                for l in range(NH)
            ]
            lhsTf = cpool.tile([P, NG], SD, tag="lhsTf")
            lhsTfN = cpool.tile([P, NG], SD, tag="lhsTfN")
            bias = cpool.tile([P, 4, 2], F32, tag="bias")
            bfh = cpool.tile([P, 1], F32, tag="bfh")
            nc.sync.dma_start(ident[:], id_d[:])
            for i in range(12):
                nc.scalar.dma_start(lhsTL[i][:], lhsTL_d[i])
            for l in range(NH):
                nc.scalar.dma_start(lhsTh[l][:], lhsTh_d[l])
                nc.scalar.dma_start(lhsThN[l][:], lhsThN_d[l])
            nc.scalar.dma_start(lhsTf[:], lhsTf_d[:])
            nc.scalar.dma_start(lhsTfN[:], lhsTfN_d[:])
            nc.sync.dma_start(bias[:], bias_d[:])
            nc.sync.dma_start(bfh[:], bfh_d[:])

            def bias_r(l, h):
                return bias[:, l, h : h + 1]

            # ---- stage A: sgrid GBM math -> S5T staging ----
            # S5T[p, x, kgp]: kgp = 4k+g; planes k: 0 t, 1 ones, 2 s, 3 Ds, 4 s*r
            S5T = sgpool.tile([P, NX, KG], SD, tag="S5T")
            rs = sgpool.tile([P, NB, NSTEP], F32, tag="rs")
            nc.sync.dma_start(rs[:], rn_d[:].rearrange("p (b n) -> p b n", b=NB))
            # constant planes (t, ones) from DRAM
            nc.sync.dma_start(
                S5T[:, :, 0:8],
                tk_d[:].rearrange("p (x k) -> p x k", k=8),
            )
            # pad rows: keep finite for the transpose passthrough
            nc.gpsimd.memset(S5T[:, :, KREAL:KG], 0.0)
            scr = sgpool.tile([P, NB, NSTEP], F32, tag="scr")
            m = sgpool.tile([P, NB, NSTEP], F32, tag="m")
            # m = (c2*r + c1)*r + c0
            nc.vector.tensor_scalar(scr[:], rs[:], c2, c1, ALU.mult, ALU.add)
            nc.vector.scalar_tensor_tensor(m[:], scr[:], 0.0, rs[:], ALU.add, ALU.mult)
            nc.vector.tensor_scalar(m[:], m[:], 1.0, c0, ALU.mult, ALU.add)

            sfull = sgpool.tile([P, NB, NSTEP + 1], F32, tag="sfull")
            nc.vector.memset(sfull[:, :, 0:1], 1.0)
            for b in range(NB):
                nc.vector.tensor_tensor_scan(
                    sfull[:, b, 1 : NSTEP + 1],
                    m[:, b, :],
                    m[:, b, :],
                    1.0,
                    ALU.mult,
                    ALU.bypass,
                )
            sN = sfull[:, :, 0:NSTEP]

            # plane views into S5T: iteration (p, g, b2, n) matching sgrid (p, b=2g+b2, n)
            def plane(k):
                return S5T[:].rearrange("p (b2 n) (k g) -> k p g b2 n", k=8, b2=2)[k]

            def sg_gb(t_ap):
                # sgrid [p, b, n] -> [p, g, b2, n]
                return t_ap.rearrange("p (g b2) n -> p g b2 n", g=NG)

            # s plane (Pool), Ds plane (DVE), s*r plane (Pool)
            nc.gpsimd.tensor_copy(plane(2), sg_gb(sN))
            nc.vector.scalar_tensor_tensor(
                plane(3), sg_gb(m[:]), 1.0, sg_gb(sN), ALU.subtract, ALU.mult
            )
            nc.gpsimd.tensor_tensor(plane(4), sg_gb(sN), sg_gb(rs[:]), ALU.mult)

            # ---- staging for stage D: rows 32s+g, cols j = x*128+p ----
            staging = sgpool.tile([P, C], SD, tag="staging")

            # ---- stage C: chunked MLP jet (wavefront-pipelined emission) ----
            # Stages per chunk c:
            #   j=0: PE transpose + DVE rhsb copy
            #   j=1..4: layer l=j-1: PE matmuls + ACT (s1, T, zu16) + elementwise
            #   j=5: final matmuls + staging copy
            # Emitting stage j of chunk k-j at iteration k keeps every engine
            # queue filled with ~6 different chunks' ready work (in-order
            # engine queues would otherwise stall on the intra-chunk chain).
            nchunk = DBG_NCHUNK or NCHUNK
            cstate = {}

            def st_transpose(c):
                pst = pstpool.tile([P, P], SD, tag="pst")
                nc.tensor.transpose(pst[:], S5T[:, 4 * c : 4 * c + 4, :], ident[:])
                rhsb = spool.tile([P, P], SD, tag="rhsb")
                if c & 1:
                    nc.vector.tensor_copy(rhsb[:], pst[:])
                else:
                    nc.scalar.activation(rhsb[:], pst[:], AF.Identity)
                cstate[c] = {"rhsb": rhsb}

            def st_layer(c, l):
                S = cstate[c]
                if l == 0:
                    Zp = pspool.tile([P, CC], F32, tag="ps")
                    Zw = pspool.tile([P, CC], F32, tag="ps")
                    Zu = pspool.tile([P, CC], F32, tag="ps")
                    rv = S.pop("rhsb")
                    for xi in range(4):
                        sl = slice(xi * P, (xi + 1) * P)
                        nc.tensor.matmul(Zp[:, sl], lhsTL[0 + xi][:], rv[:], start=True, stop=True)
                        nc.tensor.matmul(Zw[:, sl], lhsTL[4 + xi][:], rv[:], start=True, stop=True)
                        nc.tensor.matmul(Zu[:, sl], lhsTL[8 + xi][:], rv[:], start=True, stop=True)
                else:
                    a_p, u_p = S.pop("a"), S.pop("u")
                    q_p, ph_p = S.pop("q"), S.pop("ph")
                    Zp = pspool.tile([P, CC], F32, tag="ps")
                    Zu = pspool.tile([P, CC], F32, tag="ps")
                    Zw = pspool.tile([P, CC], F32, tag="ps")
                    nc.tensor.matmul(Zp[:], lhsTh[l - 1][:], a_p[:], start=True, stop=True)
                    nc.tensor.matmul(Zu[:], lhsTh[l - 1][:], u_p[:], start=True, stop=True)
                    # w = q - ph folded into the matmul: Zw = W*q + (-W)*ph
                    nc.tensor.matmul(Zw[:], lhsTh[l - 1][:], q_p[:], start=True, stop=False)
                    nc.tensor.matmul(Zw[:], lhsThN[l - 1][:], ph_p[:], start=False, stop=True)

                s1 = wpool.tile([P, CC], SD, tag="s1")
                nc.scalar.activation(
                    s1[:], Zp[:], AF.Derivative_silu, bias=bias_r(l, 0)
                )
                T = wpool.tile([P, CC], SD, tag="T")
                nc.scalar.activation(
                    T[:], Zp[:], AF.Tanh, bias=bias_r(l, 1), scale=0.5
                )
                zu16 = wpool.tile([P, CC], SD, tag="zu16")
                nc.scalar.activation(zu16[:], Zu[:], AF.Identity)

                # sigma = 0.5*T + 0.5                        (DVE ts 4x)
                sg = wpool.tile([P, CC], SD, tag="sg")
                nc.vector.tensor_scalar(sg[:], T[:], 0.5, 0.5, ALU.mult, ALU.add)
                # a' = (Zp + b) * sigma                      (DVE stt, PSUM)
                a = spool.tile([P, CC], SD, tag="a")
                nc.vector.scalar_tensor_tensor(
                    a[:], Zp[:], bias_r(l, 0), sg[:], ALU.add, ALU.mult
                )
                # q = Zw * s1                                (DVE tt, PSUM)
                q = spool.tile([P, CC], SD, tag="q")
                nc.vector.tensor_tensor(q[:], Zw[:], s1[:], ALU.mult)
                # u' = zu16 * s1                             (Pool)
                u = spool.tile([P, CC], SD, tag="u")
                nc.gpsimd.tensor_tensor(u[:], zu16[:], s1[:], ALU.mult)
                # A = zu16^2                                 (Pool)
                A = wpool.tile([P, CC], SD, tag="A")
                nc.gpsimd.tensor_tensor(A[:], zu16[:], zu16[:], ALU.mult)
                # P1 = s1*T; D = P1 - sigma = -silu''
                P1 = wpool.tile([P, CC], SD, tag="P1")
                peng = nc.vector if l < 2 else nc.gpsimd
                peng.tensor_tensor(P1[:], s1[:], T[:], ALU.mult)
                D = wpool.tile([P, CC], SD, tag="D")
                nc.gpsimd.tensor_tensor(D[:], P1[:], sg[:], ALU.subtract)
                # phi = D*A = -silu''*A  (w' = q - phi folds into next matmul)
                ph = spool.tile([P, CC], SD, tag="ph")
                nc.gpsimd.tensor_tensor(ph[:], D[:], A[:], ALU.mult)
                S["a"], S["u"], S["q"], S["ph"] = a, u, q, ph

            def st_final(c):
                S = cstate.pop(c)
                a, u, q, ph = S["a"], S["u"], S["q"], S["ph"]
                F = pspool.tile([P, CC], F32, tag="ps")
                nc.tensor.matmul(F[0:4, :], lhsTf[:], a[:], start=True, stop=True)
                nc.tensor.matmul(F[32:36, :], lhsTf[:], u[:], start=True, stop=True)
                nc.tensor.matmul(F[64:68, :], lhsTf[:], q[:], start=True, stop=False)
                nc.tensor.matmul(F[64:68, :], lhsTfN[:], ph[:], start=False, stop=True)
                dst = staging[0:68, :].rearrange("r (p x) -> r p x", p=P)[
                    :, :, 4 * c : 4 * c + 4
                ]
                fin = F[0:68, :].rearrange("r (xi p) -> r p xi", xi=4)
                nc.vector.tensor_copy(dst, fin)

            for k in range(nchunk + 6):
                for j in range(6):
                    c = k - j
                    if not (0 <= c < nchunk):
                        continue
                    if j == 0:
                        st_transpose(c)
                    elif j < 5:
                        st_layer(c, j - 1)
                    else:
                        st_final(c)

            # ---- stage D: repack + phi assembly + reduction ----
            zf_sg = sgpool.tile([P, NB, NSTEP], SD, tag="zf_sg")
            uf_sg = sgpool.tile([P, NB, NSTEP], SD, tag="uf_sg")
            wf_sg = sgpool.tile([P, NB, NSTEP], SD, tag="wf_sg")
            if DBG_NCHUNK is None:
                for s, sgt in enumerate((zf_sg, uf_sg, wf_sg)):
                    for g in range(NG):
                        row = 32 * s + g
                        src = staging[row : row + 1, :].rearrange(
                            "one (p x) -> one p x", p=P
                        )
                        dst = sgt[:, 2 * g : 2 * g + 2, :].rearrange(
                            "p b2 n -> p (b2 n)"
                        )
                        nc.sync.dma_start(dst, src)

                Tf = sgpool.tile([P, NB, NSTEP], SD, tag="Tf")
                nc.scalar.activation(Tf[:], zf_sg[:], AF.Tanh, bias=bfh[:], scale=0.5)
                U2 = sgpool.tile([P, NB, NSTEP], SD, tag="U2")
                nc.vector.scalar_tensor_tensor(
                    U2[:], uf_sg[:], 0.0, uf_sg[:], ALU.add, ALU.mult
                )
                Q = sgpool.tile([P, NB, NSTEP], SD, tag="Q")
                nc.vector.scalar_tensor_tensor(
                    Q[:], U2[:], 0.0, Tf[:], ALU.add, ALU.mult
                )
                Dd = sgpool.tile([P, NB, NSTEP], SD, tag="Dd")
                nc.vector.scalar_tensor_tensor(
                    Dd[:], wf_sg[:], 0.0, Q[:], ALU.add, ALU.subtract
                )
                T2 = sgpool.tile([P, NB, NSTEP], SD, tag="T2")
                nc.vector.scalar_tensor_tensor(
                    T2[:], Tf[:], 0.0, Tf[:], ALU.add, ALU.mult
                )
                sp = sgpool.tile([P, NB, NSTEP], SD, tag="sp")
                nc.vector.tensor_scalar(sp[:], T2[:], -0.25, 0.25, ALU.mult, ALU.add)
                Sd = sgpool.tile([P, NB, NSTEP], SD, tag="Sd")
                vT = sgpool.tile([P, NB], F32, tag="vT")
                for b in range(NB):
                    nc.vector.scalar_tensor_tensor(
                        Sd[:, b, :],
                        Dd[:, b, :],
                        0.0,
                        sp[:, b, :],
                        ALU.add,
                        ALU.mult,
                        accum_out=vT[:, b : b + 1],
                    )
            else:
                vT = sgpool.tile([P, NB], F32, tag="vT")
                nc.vector.memset(vT[:], 0.0)

            # ---- outputs ----
            yv = out_d[:].rearrange("(b p) c -> p b c", p=P)
            nc.sync.dma_start(yv[:, :, 0:1], sfull[:, :, NSTEP : NSTEP + 1])
            nc.sync.dma_start(
                yv[:, :, 1:2], vT[:].rearrange("p (b one) -> p b one", one=1)
            )

    _legalize_waits(nc)
    return nc


def _prep_host(inputs):
    rnorm = np.ascontiguousarray(np.asarray(inputs["rnorm"], dtype=np.float32))
    W0 = np.asarray(inputs["W0"], dtype=np.float32)
    b0 = np.asarray(inputs["b0"], dtype=np.float32)
    Wh = np.asarray(inputs["Wh"], dtype=np.float32)
    bh = np.asarray(inputs["bh"], dtype=np.float32)
    Wf = np.asarray(inputs["Wf"], dtype=np.float32)
    bf = np.asarray(inputs["bf"], dtype=np.float32)

    sd_np = mybir.dt.np(SD)

    # tk[p, x*8 + kgp]: t/ones planes; t = DT*(x mod 128) for every p
    tk = np.zeros((P, NX, 8), np.float32)
    n_of_x = np.tile(np.arange(NSTEP, dtype=np.float32), 2)
    for g in range(NG):
        tk[:, :, 0 + g] = (DT * n_of_x)[None, :]
        tk[:, :, 4 + g] = 1.0

    ident = np.eye(P, dtype=np.float32)

    # lhsT rows r = 4k+g; planes k: 0 t, 1 ones, 2 s, 3 Ds, 4 s*r
    # K=128 with 4 quadrant-masked variants per stream: variant xi has the
    # 32-row coef block at partitions 32*xi and zeros elsewhere
    l0 = np.zeros((KG, P), np.float32)
    lg = np.zeros((KG, P), np.float32)
    lu = np.zeros((KG, P), np.float32)
    for g in range(NG):
        cols = slice(32 * g, 32 * (g + 1))
        l0[4 * 0 + g, cols] = W0[:, 0]                          # t
        l0[4 * 2 + g, cols] = W0[:, 1]                          # s
        lg[4 * 1 + g, cols] = W0[:, 0] * DT                     # ones -> dhdt*dt
        lg[4 * 3 + g, cols] = W0[:, 1]                          # Ds
        lu[4 * 4 + g, cols] = W0[:, 1] * SIG * float(np.sqrt(0.5 * DT))
    lhsTL = np.zeros((12, P, P), np.float32)
    for s, blk in enumerate((l0, lg, lu)):
        for xi in range(4):
            lhsTL[s * 4 + xi, 32 * xi : 32 * (xi + 1), :] = blk

    lhsTh = np.zeros((NH, P, P), np.float32)
    for l in range(NH):
        for g in range(NG):
            blk = slice(32 * g, 32 * (g + 1))
            lhsTh[l, blk, blk] = Wh[l].T
    lhsTf = np.zeros((P, NG), np.float32)
    for g in range(NG):
        lhsTf[32 * g : 32 * (g + 1), g] = Wf[0]
    lhsThN = -lhsTh
    lhsTfN = -lhsTf

    bias = np.zeros((P, 4, 2), np.float32)
    bias[:, 0, 0] = np.tile(b0, NG)
    bias[:, 0, 1] = 0.5 * bias[:, 0, 0]
    for l in range(NH):
        bias[:, l + 1, 0] = np.tile(bh[l], NG)
        bias[:, l + 1, 1] = 0.5 * bias[:, l + 1, 0]
    bfh = np.full((P, 1), 0.5 * bf[0], np.float32)

    shared = {
        "tk": tk.reshape(P, NX * 8).astype(sd_np),
        "ident": ident.astype(sd_np),
        "lhsTL": lhsTL.astype(sd_np),
        "lhsTh": lhsTh.astype(sd_np),
        "lhsThN": lhsThN.astype(sd_np),
        "lhsTf": lhsTf.astype(sd_np),
        "lhsTfN": lhsTfN.astype(sd_np),
        "bias": bias,
        "bfh": bfh,
    }

    in_maps = []
    for core in range(NCORE):
        shard = rnorm[core * BC : (core + 1) * BC]          # [1024, 128]
        sg = np.ascontiguousarray(
            shard.reshape(NB, P, NSTEP).transpose(1, 0, 2).reshape(P, NB * NSTEP)
        )
        in_maps.append({"rn_sg": sg, **shared})
    return in_maps


last_perf = {}


def kernel(trace=False, **inputs) -> np.ndarray:
    if "nc" not in _CACHE:
        _CACHE["nc"] = _build_program()
    nc = _CACHE["nc"]
    in_maps = _prep_host(inputs)
    res = run_bass_kernel_spmd(nc, in_maps, list(range(NCORE)), trace=trace)
    last_perf["exec_time_ns"] = res.exec_time_ns
    out = np.empty((B, 2), np.float32)
    for core in range(NCORE):
        yt = res.results[core]["yT"]                        # [1024, 2]
        out[core * BC : (core + 1) * BC] = yt
    return out
